# revision 9
# baseline (speedup 1.0000x reference)
"""MobileViTV2 block kernel v2 — 8 TRN2 cores, data-parallel over batch.

All-bf16 datapath (fp32 PSUM), CH=1024 t-slice chunks, patch-major token
order j = p*256 + hp*16 + wp inside each chunk. z streams through DRAM in
bf16 between sweeps. Per block:
  sweep_A: normalize -> v GEMM -> relu -> spill v; zsum += cs*zn (ttr)
     [k-GEMM eliminated: cv = Wk^T zsum * (1/Z) + kB]
  sweep_B: per-patch wo GEMMs with cv FOLDED INTO wo weights; residual
     added via identity-matmul into PSUM; LN2 stats.
  sweep_F: FFN (+residual via identity-matmul) + next LN1 stats + q GEMM;
     last block variant does pw2 -> out (natural order restored in the
     PSUM->SBUF move).
LN stat math is batched: per-chunk stat rows (sum/sumsq/qraw via ones/wq
matmuls) bounce through DRAM into [16,1024] chunk-per-partition tiles so
the stat pipeline runs once per LN event, not once per chunk. R and M*R
rows bounce back via DMA broadcast reads. Conv taps split across PE
(diag-matmul), DVE and GPSIMD (env-tunable).
"""

import sys

sys.path.insert(0, "/opt/trn_rl_repo")
import os
import numpy as np
from contextlib import ExitStack

import concourse.bass as bass
import concourse.mybir as mybir
import concourse.tile as tile
from concourse import bacc
from concourse.bass_utils import run_bass_kernel_spmd

F32 = mybir.dt.float32
F32R = mybir.dt.float32r
BF16 = mybir.dt.bfloat16
AF = mybir.ActivationFunctionType
OP = mybir.AluOpType
NPBF16 = mybir.dt.np(BF16)

B, C, T, H, W = 8, 256, 16, 32, 32
D, OUTC, NBLK, FF = 384, 256, 2, 768
NTOK = T * H * W
CH = 1024
NCH = 16
PWD = 34
PSL = PWD * PWD
EPS = 1e-5

STAGE = int(os.environ.get("KERNEL_STAGE", "3"))
SUB = int(os.environ.get("KERNEL_SUB", "9"))
SIM_SAFE = bool(int(os.environ.get("KERNEL_SIM_SAFE", "0")))
N_PE_TAPS = int(os.environ.get("KERNEL_PE_TAPS", "19"))
N_GPS_TAPS = int(os.environ.get("KERNEL_GPS_TAPS", "0"))
ALL_TAPS = list(range(27))
PE_TAPS = ALL_TAPS[:N_PE_TAPS]
GPS_TAPS = ALL_TAPS[N_PE_TAPS : N_PE_TAPS + N_GPS_TAPS]
DVE_TAPS = ALL_TAPS[N_PE_TAPS + N_GPS_TAPS :]

SILU = AF.Square if SIM_SAFE else AF.Silu
EXP = AF.Square if SIM_SAFE else AF.Exp


def rawap(base, dims):
    return bass.AP(tensor=base.tensor, offset=base.offset, ap=[base.ap[0]] + dims)


def bcast_row(row, n, cols):
    """DRAM row -> [n, cols] broadcast-read AP."""
    return bass.AP(tensor=row.tensor, offset=row.offset, ap=[[0, n], [1, cols]])


# patch-major <-> natural permute views (within one 1024-token t-slice)
def nat2pm_in(ps):
    return ps.rearrange("p (hp ph wp pw) -> p ph pw hp wp", hp=16, ph=2, wp=16, pw=2)


def nat2pm_out(zslice):
    return zslice.rearrange("p (ph pw hp wp) -> p ph pw hp wp", ph=2, pw=2, hp=16, wp=16)


def pm2nat_in(ps):
    return ps.rearrange("p (ph pw hp wp) -> p ph hp wp pw", ph=2, pw=2, hp=16, wp=16)


def pm2nat_out(oslice):
    return oslice.rearrange("p (hp ph wp pw) -> p ph hp wp pw", hp=16, ph=2, wp=16, pw=2)


def build():
    nc = bacc.Bacc("TRN2", target_bir_lowering=False, debug=False, num_devices=8)

    xbf = nc.dram_tensor("xbf", [2, 128, T, H * W], BF16, kind="ExternalInput").ap()
    dwTap = nc.dram_tensor("dwTap", [2, 128, 27], F32, kind="ExternalInput").ap()
    dwBd = nc.dram_tensor("dwB", [128, 2], F32, kind="ExternalInput").ap()
    dwDiag = nc.dram_tensor("dwDiag", [max(1, len(PE_TAPS)), 2, 128, 128], BF16, kind="ExternalInput").ap()
    eyeD = nc.dram_tensor("eye", [128, 128], BF16, kind="ExternalInput").ap()
    pw1W = nc.dram_tensor("pw1W", [C, D], BF16, kind="ExternalInput").ap()
    pw1B = nc.dram_tensor("pw1B", [128, 3], F32, kind="ExternalInput").ap()
    pw2W = nc.dram_tensor("pw2W", [D, OUTC], BF16, kind="ExternalInput").ap()
    pw2B = nc.dram_tensor("pw2B", [128, 2], F32, kind="ExternalInput").ap()
    blk = []
    for i in range(NBLK):
        blk.append(dict(
            wq=nc.dram_tensor(f"wq{i}", [D, 1], BF16, kind="ExternalInput").ap(),
            wk=nc.dram_tensor(f"wk{i}", [D, D], BF16, kind="ExternalInput").ap(),
            wv=nc.dram_tensor(f"wv{i}", [D, D], BF16, kind="ExternalInput").ap(),
            qB=nc.dram_tensor(f"qB{i}", [1, 1], F32, kind="ExternalInput").ap(),
            kB=nc.dram_tensor(f"kB{i}", [128, 3], F32, kind="ExternalInput").ap(),
            vB=nc.dram_tensor(f"vB{i}", [128, 3], F32, kind="ExternalInput").ap(),
            woW=nc.dram_tensor(f"woW{i}", [D, D], BF16, kind="ExternalInput").ap(),
            woB=nc.dram_tensor(f"woB{i}", [128, 3], F32, kind="ExternalInput").ap(),
            ff1W=nc.dram_tensor(f"ff1W{i}", [D, FF], BF16, kind="ExternalInput").ap(),
            ff1B=nc.dram_tensor(f"ff1B{i}", [128, 6], F32, kind="ExternalInput").ap(),
            ff2W=nc.dram_tensor(f"ff2W{i}", [FF, D], BF16, kind="ExternalInput").ap(),
            ff2B=nc.dram_tensor(f"ff2B{i}", [128, 3], F32, kind="ExternalInput").ap(),
        ))
    qfix = nc.dram_tensor("qfix", [NBLK, 1], F32, kind="ExternalInput").ap()

    out = nc.dram_tensor("out", [OUTC, NTOK], F32, kind="ExternalOutput").ap()
    zst = [nc.dram_tensor(n, [3, 128, NTOK], BF16, kind="ExternalOutput").ap()
           for n in ("z0", "zm0", "z1", "zm1")]
    vd = [nc.dram_tensor(f"v{i}", [3, 128, NTOK], BF16).ap() for i in range(NBLK)]
    statd = [nc.dram_tensor(f"statd{e}", [3, NCH, CH], BF16).ap() for e in range(4)]
    rd = [nc.dram_tensor(f"rd{e}", [2, NCH, CH], BF16).ap() for e in range(4)]
    csd = [nc.dram_tensor(f"csd{i}", [NCH, CH], BF16).ap() for i in range(NBLK)]

    with ExitStack() as ctx:
        tc = ctx.enter_context(tile.TileContext(nc))
        wpool = ctx.enter_context(tc.tile_pool(name="w", bufs=1))
        sp = ctx.enter_context(tc.tile_pool(name="s", bufs=2))
        pp = ctx.enter_context(tc.tile_pool(name="ps", bufs=2, space="PSUM"))
        cvp = ctx.enter_context(tc.tile_pool(name="cv", bufs=1))
        p1 = ctx.enter_context(tc.tile_pool(name="p1", bufs=2))

        def wt(name, dram, kdim, mdim, dt=BF16):
            tiles = []
            for ki in range((kdim + 127) // 128):
                t = wpool.tile([128, mdim], dt, tag=f"{name}{ki}")
                nc.sync.dma_start(out=t[:], in_=dram[ki * 128 : (ki + 1) * 128, :])
                tiles.append(t)
            return tiles

        def ftile(name, dram, cols):
            t = wpool.tile([128, cols], F32, tag=name)
            nc.sync.dma_start(out=t[:], in_=dram)
            return t

        dwT = wpool.tile([128, 2, 27], F32, tag="dwT")
        for cti in range(2):
            nc.sync.dma_start(out=dwT[:, cti, :], in_=dwTap[cti])
        dwb_t = ftile("dwB", dwBd, 2)
        diag_t = None
        if PE_TAPS:
            diag_t = wpool.tile([128, len(PE_TAPS), 2, 128], BF16, tag="diag")
            for ti in range(len(PE_TAPS)):
                for cti in range(2):
                    nc.sync.dma_start(out=diag_t[:, ti, cti, :], in_=dwDiag[ti, cti])
        eye_t = wpool.tile([128, 128], BF16, tag="eye")
        nc.sync.dma_start(out=eye_t[:], in_=eyeD)
        pw1_t = wt("pw1", pw1W, C, D)
        pw1b_t = ftile("pw1B", pw1B, 3)
        pw2_t = wt("pw2", pw2W, D, OUTC)
        pw2b_t = ftile("pw2B", pw2B, 2)
        bw = []
        for i in range(NBLK):
            bw.append(dict(
                wq=wt(f"wq{i}_", blk[i]["wq"], D, 1),
                wk=wt(f"wk{i}_", blk[i]["wk"], D, D),
                wv=wt(f"wv{i}_", blk[i]["wv"], D, D),
                kB=ftile(f"kB{i}", blk[i]["kB"], 3),
                vB=ftile(f"vB{i}", blk[i]["vB"], 3),
                wo=wt(f"wo{i}_", blk[i]["woW"], D, D),
                woB=ftile(f"woB{i}", blk[i]["woB"], 3),
                ff1=wt(f"ff1{i}_", blk[i]["ff1W"], D, FF),
                ff1B=ftile(f"ff1B{i}", blk[i]["ff1B"], 6),
                ff2=wt(f"ff2{i}_", blk[i]["ff2W"], FF, D),
                ff2B=ftile(f"ff2B{i}", blk[i]["ff2B"], 3),
            ))
        ones_bf = wpool.tile([128, 1], BF16, tag="ones_bf")
        nc.vector.memset(ones_bf[:], 1.0)
        ones16 = wpool.tile([16, 1], F32, tag="ones16")
        nc.vector.memset(ones16[:], 1.0)
        ones1r = wpool.tile([1, 128], F32, tag="ones1r")
        nc.vector.memset(ones1r[:], 1.0)
        eps16 = wpool.tile([16, 1], F32, tag="eps16")
        nc.vector.memset(eps16[:], EPS)
        qb16 = [wpool.tile([16, 1], F32, tag=f"qb16_{i}", name=f"qb16_{i}") for i in range(NBLK)]
        sq16 = [wpool.tile([16, 1], F32, tag=f"sq16_{i}", name=f"sq16_{i}") for i in range(NBLK)]
        for i in range(NBLK):
            nc.sync.dma_start(out=qb16[i][:], in_=bcast_row(blk[i]["qB"][0, :], 16, 1))
            nc.sync.dma_start(out=sq16[i][:], in_=bcast_row(qfix[i, :], 16, 1))

        wop = [cvp.tile([128, 4, 384], BF16, tag=f"wop{k}", name=f"wop{k}") for k in range(3)]
        att = []
        for i in range(NBLK):
            a = dict(
                cvacc=cvp.tile([128, 3, 4], F32, tag=f"cvacc{i}", name=f"cvacc{i}"),
                zinvb=cvp.tile([128, 4], F32, tag=f"zinvb{i}", name=f"zinvb{i}"),
                wop=wop,
            )
            nc.vector.memset(a["cvacc"][:], 0.0)
            att.append(a)

        # ---------- helpers ----------
        def ln_stats(zt, wq_tiles, event, chunk, zsq_on_act):
            zsq = sp.tile([128, 3, CH], BF16, tag="zsq", name="zsq", bufs=1)
            if zsq_on_act:
                nc.scalar.activation(out=zsq[:], in_=zt[:], func=AF.Square)
            else:
                nc.vector.tensor_mul(zsq[:], zt[:], zt[:])
            ps = pp.tile([65, CH], F32, tag="pstat", bufs=1)
            for half in range(2):
                hsl = slice(half * 512, (half + 1) * 512)
                for kt in range(3):
                    nc.tensor.matmul(ps[0:1, hsl], ones_bf[:], zt[:, kt, hsl], start=(kt == 0), stop=(kt == 2))
                    nc.tensor.matmul(ps[32:33, hsl], ones_bf[:], zsq[:, kt, hsl], start=(kt == 0), stop=(kt == 2))
                    if wq_tiles is not None:
                        nc.tensor.matmul(ps[64:65, hsl], wq_tiles[kt][:], zt[:, kt, hsl], start=(kt == 0), stop=(kt == 2))
            sb = sp.tile([65, CH], BF16, tag="sbstat", name="sbstat", bufs=2)
            nc.vector.tensor_copy(sb[:], ps[:])
            nc.sync.dma_start(out=statd[event][0, chunk, :], in_=sb[0:1, :])
            nc.sync.dma_start(out=statd[event][1, chunk, :], in_=sb[32:33, :])
            if wq_tiles is not None:
                nc.sync.dma_start(out=statd[event][2, chunk, :], in_=sb[64:65, :])

        def batch_math(event, bi=None):
            with nc.allow_low_precision(reason="LN stat math in bf16 is within error budget"):
                return _batch_math(event, bi)

        def _batch_math(event, bi=None):
            bs = sp.tile([16, 3, CH], BF16, tag="bs", name="bs", bufs=1)
            for s in range(3 if bi is not None else 2):
                nc.sync.dma_start(out=bs[:, s, :], in_=statd[event][s])
            tmp = sp.tile([16, 5, CH], BF16, tag="bstmp", name="bstmp", bufs=1)
            M, t2, R, MR, q1 = (tmp[:, j, :] for j in range(5))
            nc.vector.tensor_scalar_mul(out=M, in0=bs[:, 0, :], scalar1=1.0 / D)
            nc.vector.tensor_mul(t2, M, M)
            nc.vector.scalar_tensor_tensor(out=t2, in0=bs[:, 1, :], scalar=1.0 / D, in1=t2, op0=OP.mult, op1=OP.subtract)
            nc.scalar.activation(out=t2, in_=t2, func=AF.Sqrt, bias=eps16[:])
            nc.vector.reciprocal(R, t2)
            nc.vector.tensor_mul(MR, M, R)
            rb = sp.tile([16, 2, CH], BF16, tag="rbf", name="rbf", bufs=1)
            nc.vector.tensor_copy(rb[:, 0, :], R)
            nc.vector.tensor_copy(rb[:, 1, :], MR)
            nc.sync.dma_start(out=rd[event][0], in_=rb[:, 0, :])
            nc.sync.dma_start(out=rd[event][1], in_=rb[:, 1, :])
            if bi is None:
                return
            nc.vector.scalar_tensor_tensor(out=q1, in0=M, scalar=sq16[bi][:], in1=bs[:, 2, :], op0=OP.mult, op1=OP.add)
            nc.vector.tensor_mul(q1, q1, R)
            cs = sp.tile([16, CH], BF16, tag="bscs", name="bscs", bufs=1)
            zp = sp.tile([16, 4], F32, tag="bszp", name="bszp", bufs=1)
            for p in range(4):
                nc.scalar.activation(out=cs[:, p * 256 : (p + 1) * 256], in_=q1[:, p * 256 : (p + 1) * 256],
                                     func=EXP, bias=qb16[bi][:], accum_out=zp[:, p : p + 1])
            nc.sync.dma_start(out=csd[bi], in_=cs[:])
            psz = pp.tile([128, 4], F32, tag="pstiny", bufs=1)
            nc.tensor.matmul(psz[0:1, :], ones16[:], zp[:], start=True, stop=True)
            zi = sp.tile([1, 4], F32, tag="zi", name="zi", bufs=1)
            nc.vector.reciprocal(zi[:], psz[0:1, :])
            psb = pp.tile([128, 4], F32, tag="pstiny", bufs=1)
            nc.tensor.matmul(psb[:], ones1r[:], zi[:], start=True, stop=True)
            nc.vector.tensor_copy(att[bi]["zinvb"][:], psb[:])

        def load_bcast(dram_row):
            t = sp.tile([128, CH], BF16, tag="bcast", name="bcast", bufs=4)
            nc.sync.dma_start(out=t[:], in_=bcast_row(dram_row, 128, CH))
            return t

        def load_z(dram, chunk, tag="zch"):
            zt = sp.tile([128, 3, CH], BF16, tag=tag, name=tag, bufs=2)
            for m in range(3):
                nc.sync.dma_start(out=zt[:, m, :], in_=dram[m, :, chunk * CH : (chunk + 1) * CH])
            return zt

        def normalize(zt, event, chunk, gps=False):
            rb = load_bcast(rd[event][0, chunk, :])
            mrb = load_bcast(rd[event][1, chunk, :])
            zn = sp.tile([128, 3, CH], BF16, tag="zn", name="zn", bufs=2)
            nc.vector.tensor_mul(zn[:], zt[:], rawap(rb[:], [[0, 3], [1, CH]]))
            eng = nc.gpsimd if gps else nc.vector
            eng.tensor_sub(zn[:], zn[:], rawap(mrb[:], [[0, 3], [1, CH]]))
            return zn

        # ================= S0: conv + pw1 + LN1_0 stats + q0 =================
        xslices = {}
        # zero the 3 rotating xps buffers once; interiors are overwritten by
        # each slice DMA, pad borders stay zero across rotations
        for _i in range(3):
            xz = p1.tile([128, 2, PSL], BF16, tag="xps", name="xps", bufs=3)
            nc.vector.memset(xz[:], 0.0)

        def load_slice(ts_):
            xs = p1.tile([128, 2, PSL], BF16, tag="xps", name="xps", bufs=3)
            for cti in range(2):
                dst = xs[:, cti, :].rearrange("p (h w) -> p h w", h=PWD)
                nc.sync.dma_start(out=dst[:, 1:33, 1:33], in_=xbf[cti, :, ts_, :].rearrange("p (h w) -> p h w", h=H))
            xslices[ts_] = xs

        for t in range(T):
            for ts_ in (t - 1, t, t + 1):
                if 0 <= ts_ < T and ts_ not in xslices:
                    load_slice(ts_)
            yact = p1.tile([128, 2, CH], BF16, tag="yact", name="yact")
            for cti in range(2):
                ok = lambda tp: 0 <= t + tp // 9 - 1 < T
                pe_here = [tp for tp in PE_TAPS if ok(tp)]
                dve_here = [tp for tp in DVE_TAPS if ok(tp)]
                gps_here = [tp for tp in GPS_TAPS if ok(tp)]

                def xsrc(tp):
                    dt_, dh, dw = tp // 9, (tp % 9) // 3, tp % 3
                    xv = xslices[t + dt_ - 1][:, cti, :].rearrange("p (h w) -> p h w", h=PWD)
                    return xv[:, dh : dh + 32, dw : dw + 32]

                ps_c = None
                if pe_here:
                    ps_c = pp.tile([128, CH], F32, tag="psA")
                    for half in range(2):
                        for j, tp in enumerate(pe_here):
                            dt_, dh, dw = tp // 9, (tp % 9) // 3, tp % 3
                            xv = xslices[t + dt_ - 1][:, cti, :].rearrange("p (h w) -> p h w", h=PWD)
                            nc.tensor.matmul(ps_c[:, half * 512 : (half + 1) * 512],
                                             diag_t[:, PE_TAPS.index(tp), cti, :],
                                             xv[:, dh + 16 * half : dh + 16 * half + 16, dw : dw + 32],
                                             start=(j == 0), stop=(j == len(pe_here) - 1))
                silu_src = ps_c[:] if ps_c is not None else None
                if dve_here or gps_here:
                    acc = p1.tile([128, CH], F32, tag="cacc", name="cacc", bufs=2)
                    accv = acc[:].rearrange("p (h w) -> p h w", h=H)
                    first = True
                    for tp in dve_here:
                        wcol = dwT[:, cti, tp : tp + 1]
                        if first:
                            nc.vector.tensor_scalar_mul(out=accv, in0=xsrc(tp), scalar1=wcol)
                        else:
                            nc.vector.scalar_tensor_tensor(out=accv, in0=xsrc(tp), scalar=wcol, in1=accv, op0=OP.mult, op1=OP.add)
                        first = False
                    gacc = None
                    for gj, tp in enumerate(gps_here):
                        wcol = dwT[:, cti, tp : tp + 1]
                        if gj == 0:
                            gacc = p1.tile([128, CH], F32, tag="gacc", name="gacc", bufs=1)
                            gaccv = gacc[:].rearrange("p (h w) -> p h w", h=H)
                            nc.gpsimd.tensor_scalar_mul(out=gaccv, in0=xsrc(tp), scalar1=wcol)
                        else:
                            nc.gpsimd.scalar_tensor_tensor(out=gaccv, in0=xsrc(tp), scalar=wcol, in1=gaccv, op0=OP.mult, op1=OP.add)
                    if gacc is not None and not first:
                        nc.vector.tensor_add(acc[:], acc[:], gacc[:])
                    elif gacc is not None:
                        acc = gacc
                    if ps_c is not None:
                        nc.vector.scalar_tensor_tensor(out=acc[:], in0=ps_c[:], scalar=1.0, in1=acc[:], op0=OP.mult, op1=OP.add)
                    silu_src = acc[:]
                nc.scalar.activation(out=yact[:, cti, :], in_=silu_src, func=SILU, bias=dwb_t[:, cti : cti + 1])
            zt = sp.tile([128, 3, CH], BF16, tag="zch", name="zch", bufs=2)
            for m in range(3):
                ps1 = pp.tile([128, CH], F32, tag="psA")
                for half in range(2):
                    hsl = slice(half * 512, (half + 1) * 512)
                    for kt in range(2):
                        nc.tensor.matmul(ps1[:, hsl], pw1_t[kt][:, m * 128 : (m + 1) * 128], yact[:, kt, hsl], start=(kt == 0), stop=(kt == 1))
                for ph_ in range(2):
                    nc.scalar.activation(out=nat2pm_out(zt[:, m, :])[:, ph_], in_=nat2pm_in(ps1[:])[:, ph_], func=AF.Identity, bias=pw1b_t[:, m : m + 1])
                nc.sync.dma_start(out=zst[0][m, :, t * CH : (t + 1) * CH], in_=zt[:, m, :])
            ln_stats(zt, bw[0]["wq"], 0, t, zsq_on_act=True)

        # ================= per-block sweeps =================
        def sweep_A(bi, zsrc, event):
            a = att[bi]
            for chunk in range(NCH):
                zt = load_z(zsrc, chunk)
                zn = normalize(zt, event, chunk, gps=True)
                csb = load_bcast(csd[bi][chunk, :])
                vt = sp.tile([128, 3, CH], BF16, tag="vch", name="vch", bufs=2)
                for m in range(3):
                    psv = pp.tile([128, CH], F32, tag="psA")
                    for half in range(2):
                        hsl = slice(half * 512, (half + 1) * 512)
                        for kt in range(3):
                            nc.tensor.matmul(psv[:, hsl], bw[bi]["wv"][kt][:, m * 128 : (m + 1) * 128], zn[:, kt, hsl], start=(kt == 0), stop=(kt == 2))
                    nc.scalar.activation(out=vt[:, m, :], in_=psv[:], func=AF.Relu, bias=bw[bi]["vB"][:, m : m + 1])
                    nc.sync.dma_start(out=vd[bi][m, :, chunk * CH : (chunk + 1) * CH], in_=vt[:, m, :])
                junk = sp.tile([128, 3, CH], BF16, tag="junk", name="junk", bufs=1)
                csb3 = rawap(csb[:], [[0, 3], [1, CH]])
                nc.vector.tensor_mul(junk[:], zn[:], csb3)
                cvch = sp.tile([128, 3, 4], F32, tag="cvch", name="cvch", bufs=2)
                for m in range(3):
                    for p in range(4):
                        if (m + p) % 2 == 0:
                            nc.vector.tensor_reduce(cvch[:, m, p : p + 1], junk[:, m, p * 256 : (p + 1) * 256],
                                                    axis=mybir.AxisListType.X, op=OP.add)
                        else:
                            nc.scalar.activation(out=junk[:, m, p * 256 : (p + 1) * 256], in_=junk[:, m, p * 256 : (p + 1) * 256],
                                                 func=AF.Copy, accum_out=cvch[:, m, p : p + 1])
                nc.vector.tensor_add(a["cvacc"][:], a["cvacc"][:], cvch[:])

        def finalize_cv(bi):
            a = att[bi]
            cvb = sp.tile([128, 3, 4], BF16, tag="cvb", name="cvb", bufs=1)
            nc.vector.tensor_copy(cvb[:], a["cvacc"][:])
            cvf = sp.tile([128, 3, 4], F32, tag="cvf", name="cvf", bufs=1)
            for m in range(3):
                psc = pp.tile([128, 4], F32, tag="pstiny", bufs=1)
                for kt in range(3):
                    nc.tensor.matmul(psc[:], bw[bi]["wk"][kt][:, m * 128 : (m + 1) * 128], cvb[:, kt, :], start=(kt == 0), stop=(kt == 2))
                nc.vector.scalar_tensor_tensor(out=cvf[:, m, :], in0=psc[:], scalar=bw[bi]["kB"][:, m : m + 1], in1=a["zinvb"][:], op0=OP.add, op1=OP.mult)
            for kt in range(3):
                for p in range(4):
                    nc.vector.tensor_scalar_mul(out=a["wop"][kt][:, p, :], in0=bw[bi]["wo"][kt][:], scalar1=cvf[:, kt, p : p + 1])

        def sweep_B(bi, zsrc, zdst, event):
            a = att[bi]
            for chunk in range(NCH):
                zt = load_z(zsrc, chunk)
                vt = sp.tile([128, 3, CH], BF16, tag="vch", name="vch", bufs=2)
                for m in range(3):
                    nc.sync.dma_start(out=vt[:, m, :], in_=vd[bi][m, :, chunk * CH : (chunk + 1) * CH])
                zm = sp.tile([128, 3, CH], BF16, tag="zm", name="zm", bufs=2)
                for m in range(3):
                    pso = pp.tile([128, CH], F32, tag="psA")
                    for p in range(4):
                        sl = slice(p * 256, (p + 1) * 256)
                        for kt in range(3):
                            nc.tensor.matmul(pso[:, sl], a["wop"][kt][:, p, m * 128 : (m + 1) * 128], vt[:, kt, sl], start=(kt == 0), stop=False)
                        nc.tensor.matmul(pso[:, sl], eye_t[:], zt[:, m, sl], start=False, stop=True)
                    nc.scalar.activation(out=zm[:, m, :], in_=pso[:], func=AF.Identity, bias=bw[bi]["woB"][:, m : m + 1])
                    nc.sync.dma_start(out=zdst[m, :, chunk * CH : (chunk + 1) * CH], in_=zm[:, m, :])
                ln_stats(zm, None, event, chunk, zsq_on_act=False)

        def sweep_F(bi, zsrc, zdst, event_in, event_out, wq_next, last):
            for chunk in range(NCH):
                zt = load_z(zsrc, chunk)
                zn = normalize(zt, event_in, chunk)
                z2 = sp.tile([128, 3, CH], BF16, tag="z2", name="z2", bufs=2)
                ht = sp.tile([128, 6, CH], BF16, tag="ht", name="ht", bufs=1)
                for m6 in range(6):
                    ps1 = pp.tile([128, CH], F32, tag="psA")
                    for half in range(2):
                        hsl = slice(half * 512, (half + 1) * 512)
                        for kt in range(3):
                            nc.tensor.matmul(ps1[:, hsl], bw[bi]["ff1"][kt][:, m6 * 128 : (m6 + 1) * 128], zn[:, kt, hsl], start=(kt == 0), stop=(kt == 2))
                    nc.scalar.activation(out=ht[:, m6, :], in_=ps1[:], func=SILU, bias=bw[bi]["ff1B"][:, m6 : m6 + 1])
                for m in range(3):
                    psf = pp.tile([128, CH], F32, tag="psA")
                    for half in range(2):
                        hsl = slice(half * 512, (half + 1) * 512)
                        for m6 in range(6):
                            nc.tensor.matmul(psf[:, hsl], bw[bi]["ff2"][m6][:, m * 128 : (m + 1) * 128], ht[:, m6, hsl], start=(m6 == 0), stop=False)
                        nc.tensor.matmul(psf[:, hsl], eye_t[:], zt[:, m, hsl], start=False, stop=True)
                    nc.scalar.activation(out=z2[:, m, :], in_=psf[:], func=AF.Identity, bias=bw[bi]["ff2B"][:, m : m + 1])
                    if zdst is not None:
                        nc.sync.dma_start(out=zdst[m, :, chunk * CH : (chunk + 1) * CH], in_=z2[:, m, :])
                if not last:
                    ln_stats(z2, wq_next, event_out, chunk, zsq_on_act=False)
                else:
                    for m in range(2):
                        ot = sp.tile([128, CH], F32, tag="ot", name="ot", bufs=1)
                        ps2 = pp.tile([128, CH], F32, tag="psA")
                        for half in range(2):
                            hsl = slice(half * 512, (half + 1) * 512)
                            for kt in range(3):
                                nc.tensor.matmul(ps2[:, hsl], pw2_t[kt][:, m * 128 : (m + 1) * 128], z2[:, kt, hsl], start=(kt == 0), stop=(kt == 2))
                        for ph_ in range(2):
                            nc.scalar.activation(out=pm2nat_out(ot[:])[:, ph_], in_=pm2nat_in(ps2[:])[:, ph_], func=AF.Identity, bias=pw2b_t[:, m : m + 1])
                        nc.sync.dma_start(out=out[m * 128 : (m + 1) * 128, chunk * CH : (chunk + 1) * CH], in_=ot[:])

        if STAGE >= 2:
            batch_math(0, bi=0)
            if SUB >= 2:
                sweep_A(0, zst[0], 0)
            if SUB >= 3:
                finalize_cv(0)
            if SUB >= 4:
                sweep_B(0, zst[0], zst[1], 1)
            if SUB >= 5:
                batch_math(1)
                sweep_F(0, zst[1], zst[2], 1, 2, bw[1]["wq"], last=False)
        if STAGE >= 3:
            batch_math(2, bi=1)
            sweep_A(1, zst[2], 2)
            finalize_cv(1)
            sweep_B(1, zst[2], zst[3], 3)
            batch_math(3)
            sweep_F(1, zst[3], None, 3, None, None, last=True)

    nc.compile()
    return nc


_NC = None


def _get_nc():
    global _NC
    if _NC is None:
        _NC = build()
    return _NC


def _prep(inputs):
    f32 = lambda a: np.ascontiguousarray(np.asarray(a, np.float32))
    bf = lambda a: np.ascontiguousarray(np.asarray(a, np.float32)).astype(NPBF16)
    dw = f32(inputs["dw_w"]).reshape(C, 27)
    base = {
        "dwTap": np.ascontiguousarray(dw.reshape(2, 128, 27)),
        "dwB": np.ascontiguousarray(f32(inputs["dw_b"]).reshape(2, 128).T),
        "eye": np.eye(128, dtype=np.float32).astype(NPBF16),
        "pw1W": bf(inputs["pw1_w"]),
        "pw1B": np.ascontiguousarray(f32(inputs["pw1_b"]).reshape(3, 128).T),
        "pw2W": bf(inputs["pw2_w"]),
        "pw2B": np.ascontiguousarray(f32(inputs["pw2_b"]).reshape(2, 128).T),
    }
    diag = np.zeros((max(1, len(PE_TAPS)), 2, 128, 128), np.float32)
    for ti, tp in enumerate(PE_TAPS):
        for cti in range(2):
            np.fill_diagonal(diag[ti, cti], dw[cti * 128 : (cti + 1) * 128, tp])
    base["dwDiag"] = diag.astype(NPBF16)
    qf = np.zeros((NBLK, 1), np.float32)
    for i in range(NBLK):
        qkvW = f32(inputs["ln1_g"][i])[:, None] * f32(inputs["qkv_w"][i])
        qkvB = f32(inputs["ln1_b"][i]) @ f32(inputs["qkv_w"][i]) + f32(inputs["qkv_b"][i])
        ff1W = f32(inputs["ln2_g"][i])[:, None] * f32(inputs["ff1_w"][i])
        ff1B = f32(inputs["ln2_b"][i]) @ f32(inputs["ff1_w"][i]) + f32(inputs["ff1_b"][i])
        wqb = np.ascontiguousarray(qkvW[:, 0:1]).astype(NPBF16)
        qf[i, 0] = -float(np.asarray(wqb, np.float32).sum())
        base.update({
            f"wq{i}": wqb,
            f"wk{i}": np.ascontiguousarray(qkvW[:, 1 : 1 + D]).astype(NPBF16),
            f"wv{i}": np.ascontiguousarray(qkvW[:, 1 + D :]).astype(NPBF16),
            f"qB{i}": np.ascontiguousarray(qkvB[0:1].reshape(1, 1)),
            f"kB{i}": np.ascontiguousarray(qkvB[1 : 1 + D].reshape(3, 128).T),
            f"vB{i}": np.ascontiguousarray(qkvB[1 + D :].reshape(3, 128).T),
            f"woW{i}": bf(inputs["wo_w"][i]),
            f"woB{i}": np.ascontiguousarray(f32(inputs["wo_b"][i]).reshape(3, 128).T),
            f"ff1W{i}": ff1W.astype(NPBF16),
            f"ff1B{i}": np.ascontiguousarray(ff1B.reshape(6, 128).T),
            f"ff2W{i}": bf(inputs["ff2_w"][i]),
            f"ff2B{i}": np.ascontiguousarray(f32(inputs["ff2_b"][i]).reshape(3, 128).T),
        })
    base["qfix"] = qf
    return base


def kernel(**inputs):
    base = _prep(inputs)
    x = np.asarray(inputs["x"], np.float32)
    in_maps = []
    for b in range(B):
        xb = np.ascontiguousarray(x[b].reshape(2, 128, T, H * W)).astype(NPBF16)
        in_maps.append(dict(base, xbf=xb))
    nc = _get_nc()
    trace = bool(int(os.environ.get("KERNEL_TRACE", "0")))
    res = run_bass_kernel_spmd(nc, in_maps, list(range(B)), trace=trace)
    kernel.last_exec_ns = res.exec_time_ns
    kernel.last_profile = res.profile_json
    kernel.last_results = res.results
    outs = [res.results[b]["out"].reshape(OUTC, T, H, W) for b in range(B)]
    return np.stack(outs).astype(np.float32)


# revision 10
# speedup vs baseline: 1.0447x; 1.0447x over previous
"""MobileViTV2 block kernel v2 — 8 TRN2 cores, data-parallel over batch.

All-bf16 datapath (fp32 PSUM), CH=1024 t-slice chunks, patch-major token
order j = p*256 + hp*16 + wp inside each chunk. z streams through DRAM in
bf16 between sweeps. Per block:
  sweep_A: normalize -> v GEMM -> relu -> spill v; zsum += cs*zn (ttr)
     [k-GEMM eliminated: cv = Wk^T zsum * (1/Z) + kB]
  sweep_B: per-patch wo GEMMs with cv FOLDED INTO wo weights; residual
     added via identity-matmul into PSUM; LN2 stats.
  sweep_F: FFN (+residual via identity-matmul) + next LN1 stats + q GEMM;
     last block variant does pw2 -> out (natural order restored in the
     PSUM->SBUF move).
LN stat math is batched: per-chunk stat rows (sum/sumsq/qraw via ones/wq
matmuls) bounce through DRAM into [16,1024] chunk-per-partition tiles so
the stat pipeline runs once per LN event, not once per chunk. R and M*R
rows bounce back via DMA broadcast reads. Conv taps split across PE
(diag-matmul), DVE and GPSIMD (env-tunable).
"""

import sys

sys.path.insert(0, "/opt/trn_rl_repo")
import os
import numpy as np
from contextlib import ExitStack

import concourse.bass as bass
import concourse.mybir as mybir
import concourse.tile as tile
from concourse import bacc
from concourse.bass_utils import run_bass_kernel_spmd

F32 = mybir.dt.float32
F32R = mybir.dt.float32r
BF16 = mybir.dt.bfloat16
AF = mybir.ActivationFunctionType
OP = mybir.AluOpType
NPBF16 = mybir.dt.np(BF16)

B, C, T, H, W = 8, 256, 16, 32, 32
D, OUTC, NBLK, FF = 384, 256, 2, 768
NTOK = T * H * W
CH = 1024
NCH = 16
PWD = 34
PSL = PWD * PWD
EPS = 1e-5

STAGE = int(os.environ.get("KERNEL_STAGE", "3"))
SUB = int(os.environ.get("KERNEL_SUB", "9"))
SIM_SAFE = bool(int(os.environ.get("KERNEL_SIM_SAFE", "0")))
N_PE_TAPS = int(os.environ.get("KERNEL_PE_TAPS", "27"))
N_GPS_TAPS = int(os.environ.get("KERNEL_GPS_TAPS", "0"))
ALL_TAPS = list(range(27))
PE_TAPS = ALL_TAPS[:N_PE_TAPS]
GPS_TAPS = ALL_TAPS[N_PE_TAPS : N_PE_TAPS + N_GPS_TAPS]
DVE_TAPS = ALL_TAPS[N_PE_TAPS + N_GPS_TAPS :]

SILU = AF.Square if SIM_SAFE else AF.Silu
EXP = AF.Square if SIM_SAFE else AF.Exp


def rawap(base, dims):
    return bass.AP(tensor=base.tensor, offset=base.offset, ap=[base.ap[0]] + dims)


def bcast_row(row, n, cols):
    """DRAM row -> [n, cols] broadcast-read AP."""
    return bass.AP(tensor=row.tensor, offset=row.offset, ap=[[0, n], [1, cols]])


# patch-major <-> natural permute views (within one 1024-token t-slice)
def nat2pm_in(ps):
    return ps.rearrange("p (hp ph wp pw) -> p ph pw hp wp", hp=16, ph=2, wp=16, pw=2)


def nat2pm_out(zslice):
    return zslice.rearrange("p (ph pw hp wp) -> p ph pw hp wp", ph=2, pw=2, hp=16, wp=16)


def pm2nat_in(ps):
    return ps.rearrange("p (ph pw hp wp) -> p ph hp wp pw", ph=2, pw=2, hp=16, wp=16)


def pm2nat_out(oslice):
    return oslice.rearrange("p (hp ph wp pw) -> p ph hp wp pw", hp=16, ph=2, wp=16, pw=2)


def build():
    nc = bacc.Bacc("TRN2", target_bir_lowering=False, debug=False, num_devices=8)

    xbf = nc.dram_tensor("xbf", [2, 128, T, H * W], BF16, kind="ExternalInput").ap()
    dwTap = nc.dram_tensor("dwTap", [2, 128, 27], F32, kind="ExternalInput").ap()
    dwBd = nc.dram_tensor("dwB", [128, 2], F32, kind="ExternalInput").ap()
    dwDiag = nc.dram_tensor("dwDiag", [max(1, len(PE_TAPS)), 2, 128, 128], BF16, kind="ExternalInput").ap()
    eyeD = nc.dram_tensor("eye", [128, 128], BF16, kind="ExternalInput").ap()
    pw1W = nc.dram_tensor("pw1W", [C, D], BF16, kind="ExternalInput").ap()
    pw1B = nc.dram_tensor("pw1B", [128, 3], F32, kind="ExternalInput").ap()
    pw2W = nc.dram_tensor("pw2W", [D, OUTC], BF16, kind="ExternalInput").ap()
    pw2B = nc.dram_tensor("pw2B", [128, 2], F32, kind="ExternalInput").ap()
    blk = []
    for i in range(NBLK):
        blk.append(dict(
            wq=nc.dram_tensor(f"wq{i}", [D, 1], BF16, kind="ExternalInput").ap(),
            wk=nc.dram_tensor(f"wk{i}", [D, D], BF16, kind="ExternalInput").ap(),
            wv=nc.dram_tensor(f"wv{i}", [D, D], BF16, kind="ExternalInput").ap(),
            qB=nc.dram_tensor(f"qB{i}", [1, 1], F32, kind="ExternalInput").ap(),
            kB=nc.dram_tensor(f"kB{i}", [128, 3], F32, kind="ExternalInput").ap(),
            vB=nc.dram_tensor(f"vB{i}", [128, 3], F32, kind="ExternalInput").ap(),
            woW=nc.dram_tensor(f"woW{i}", [D, D], BF16, kind="ExternalInput").ap(),
            woB=nc.dram_tensor(f"woB{i}", [128, 3], F32, kind="ExternalInput").ap(),
            ff1W=nc.dram_tensor(f"ff1W{i}", [D, FF], BF16, kind="ExternalInput").ap(),
            ff1B=nc.dram_tensor(f"ff1B{i}", [128, 6], F32, kind="ExternalInput").ap(),
            ff2W=nc.dram_tensor(f"ff2W{i}", [FF, D], BF16, kind="ExternalInput").ap(),
            ff2B=nc.dram_tensor(f"ff2B{i}", [128, 3], F32, kind="ExternalInput").ap(),
        ))
    qfix = nc.dram_tensor("qfix", [NBLK, 1], F32, kind="ExternalInput").ap()

    out = nc.dram_tensor("out", [OUTC, NTOK], F32, kind="ExternalOutput").ap()
    zst = [nc.dram_tensor(n, [3, 128, NTOK], BF16, kind="ExternalOutput").ap()
           for n in ("z0", "zm0", "z1", "zm1")]
    vd = [nc.dram_tensor(f"v{i}", [3, 128, NTOK], BF16).ap() for i in range(NBLK)]
    statd = [nc.dram_tensor(f"statd{e}", [3, NCH, CH], BF16).ap() for e in range(4)]
    rd = [nc.dram_tensor(f"rd{e}", [2, NCH, CH], BF16).ap() for e in range(4)]
    csd = [nc.dram_tensor(f"csd{i}", [NCH, CH], BF16).ap() for i in range(NBLK)]

    with ExitStack() as ctx:
        tc = ctx.enter_context(tile.TileContext(nc))
        wpool = ctx.enter_context(tc.tile_pool(name="w", bufs=1))
        sp = ctx.enter_context(tc.tile_pool(name="s", bufs=2))
        pp = ctx.enter_context(tc.tile_pool(name="ps", bufs=2, space="PSUM"))
        cvp = ctx.enter_context(tc.tile_pool(name="cv", bufs=1))
        p1 = ctx.enter_context(tc.tile_pool(name="p1", bufs=2))

        def wt(name, dram, kdim, mdim, dt=BF16):
            tiles = []
            for ki in range((kdim + 127) // 128):
                t = wpool.tile([128, mdim], dt, tag=f"{name}{ki}")
                nc.sync.dma_start(out=t[:], in_=dram[ki * 128 : (ki + 1) * 128, :])
                tiles.append(t)
            return tiles

        def ftile(name, dram, cols):
            t = wpool.tile([128, cols], F32, tag=name)
            nc.sync.dma_start(out=t[:], in_=dram)
            return t

        dwT = wpool.tile([128, 2, 27], F32, tag="dwT")
        for cti in range(2):
            nc.sync.dma_start(out=dwT[:, cti, :], in_=dwTap[cti])
        dwb_t = ftile("dwB", dwBd, 2)
        diag_t = None
        if PE_TAPS:
            diag_t = wpool.tile([128, len(PE_TAPS), 2, 128], BF16, tag="diag")
            for ti in range(len(PE_TAPS)):
                for cti in range(2):
                    nc.sync.dma_start(out=diag_t[:, ti, cti, :], in_=dwDiag[ti, cti])
        eye_t = wpool.tile([128, 128], BF16, tag="eye")
        nc.sync.dma_start(out=eye_t[:], in_=eyeD)
        pw1_t = wt("pw1", pw1W, C, D)
        pw1b_t = ftile("pw1B", pw1B, 3)
        pw2_t = wt("pw2", pw2W, D, OUTC)
        pw2b_t = ftile("pw2B", pw2B, 2)
        bw = []
        for i in range(NBLK):
            bw.append(dict(
                wq=wt(f"wq{i}_", blk[i]["wq"], D, 1),
                wk=wt(f"wk{i}_", blk[i]["wk"], D, D),
                wv=wt(f"wv{i}_", blk[i]["wv"], D, D),
                kB=ftile(f"kB{i}", blk[i]["kB"], 3),
                vB=ftile(f"vB{i}", blk[i]["vB"], 3),
                wo=wt(f"wo{i}_", blk[i]["woW"], D, D),
                woB=ftile(f"woB{i}", blk[i]["woB"], 3),
                ff1=wt(f"ff1{i}_", blk[i]["ff1W"], D, FF),
                ff1B=ftile(f"ff1B{i}", blk[i]["ff1B"], 6),
                ff2=wt(f"ff2{i}_", blk[i]["ff2W"], FF, D),
                ff2B=ftile(f"ff2B{i}", blk[i]["ff2B"], 3),
            ))
        ones_bf = wpool.tile([128, 1], BF16, tag="ones_bf")
        nc.vector.memset(ones_bf[:], 1.0)
        ones16 = wpool.tile([16, 1], F32, tag="ones16")
        nc.vector.memset(ones16[:], 1.0)
        ones1r = wpool.tile([1, 128], F32, tag="ones1r")
        nc.vector.memset(ones1r[:], 1.0)
        eps16 = wpool.tile([16, 1], F32, tag="eps16")
        nc.vector.memset(eps16[:], EPS)
        qb16 = [wpool.tile([16, 1], F32, tag=f"qb16_{i}", name=f"qb16_{i}") for i in range(NBLK)]
        sq16 = [wpool.tile([16, 1], F32, tag=f"sq16_{i}", name=f"sq16_{i}") for i in range(NBLK)]
        for i in range(NBLK):
            nc.sync.dma_start(out=qb16[i][:], in_=bcast_row(blk[i]["qB"][0, :], 16, 1))
            nc.sync.dma_start(out=sq16[i][:], in_=bcast_row(qfix[i, :], 16, 1))

        wop = [cvp.tile([128, 4, 384], BF16, tag=f"wop{k}", name=f"wop{k}") for k in range(3)]
        att = []
        for i in range(NBLK):
            a = dict(
                cvacc=cvp.tile([128, 3, 4], F32, tag=f"cvacc{i}", name=f"cvacc{i}"),
                zinvb=cvp.tile([128, 4], F32, tag=f"zinvb{i}", name=f"zinvb{i}"),
                wop=wop,
            )
            nc.vector.memset(a["cvacc"][:], 0.0)
            att.append(a)

        # ---------- helpers ----------
        def ln_stats(zt, wq_tiles, event, chunk, zsq_on_act):
            zsq = sp.tile([128, 3, CH], BF16, tag="zsq", name="zsq", bufs=1)
            if zsq_on_act:
                nc.scalar.activation(out=zsq[:], in_=zt[:], func=AF.Square)
            else:
                nc.vector.tensor_mul(zsq[:], zt[:], zt[:])
            ps = pp.tile([65, CH], F32, tag="pstat", bufs=1)
            for half in range(2):
                hsl = slice(half * 512, (half + 1) * 512)
                for kt in range(3):
                    nc.tensor.matmul(ps[0:1, hsl], ones_bf[:], zt[:, kt, hsl], start=(kt == 0), stop=(kt == 2))
                    nc.tensor.matmul(ps[32:33, hsl], ones_bf[:], zsq[:, kt, hsl], start=(kt == 0), stop=(kt == 2))
                    if wq_tiles is not None:
                        nc.tensor.matmul(ps[64:65, hsl], wq_tiles[kt][:], zt[:, kt, hsl], start=(kt == 0), stop=(kt == 2))
            sb = sp.tile([65, CH], BF16, tag="sbstat", name="sbstat", bufs=2)
            nc.vector.tensor_copy(sb[:], ps[:])
            nc.sync.dma_start(out=statd[event][0, chunk, :], in_=sb[0:1, :])
            nc.sync.dma_start(out=statd[event][1, chunk, :], in_=sb[32:33, :])
            if wq_tiles is not None:
                nc.sync.dma_start(out=statd[event][2, chunk, :], in_=sb[64:65, :])

        def batch_math(event, bi=None):
            with nc.allow_low_precision(reason="LN stat math in bf16 is within error budget"):
                return _batch_math(event, bi)

        def _batch_math(event, bi=None):
            bs = sp.tile([16, 3, CH], BF16, tag="bs", name="bs", bufs=1)
            for s in range(3 if bi is not None else 2):
                nc.sync.dma_start(out=bs[:, s, :], in_=statd[event][s])
            tmp = sp.tile([16, 5, CH], BF16, tag="bstmp", name="bstmp", bufs=1)
            M, t2, R, MR, q1 = (tmp[:, j, :] for j in range(5))
            nc.vector.tensor_scalar_mul(out=M, in0=bs[:, 0, :], scalar1=1.0 / D)
            nc.vector.tensor_mul(t2, M, M)
            nc.vector.scalar_tensor_tensor(out=t2, in0=bs[:, 1, :], scalar=1.0 / D, in1=t2, op0=OP.mult, op1=OP.subtract)
            nc.scalar.activation(out=t2, in_=t2, func=AF.Sqrt, bias=eps16[:])
            nc.vector.reciprocal(R, t2)
            nc.vector.tensor_mul(MR, M, R)
            rb = sp.tile([16, 2, CH], BF16, tag="rbf", name="rbf", bufs=1)
            nc.vector.tensor_copy(rb[:, 0, :], R)
            nc.vector.tensor_copy(rb[:, 1, :], MR)
            nc.sync.dma_start(out=rd[event][0], in_=rb[:, 0, :])
            nc.sync.dma_start(out=rd[event][1], in_=rb[:, 1, :])
            if bi is None:
                return
            nc.vector.scalar_tensor_tensor(out=q1, in0=M, scalar=sq16[bi][:], in1=bs[:, 2, :], op0=OP.mult, op1=OP.add)
            nc.vector.tensor_mul(q1, q1, R)
            cs = sp.tile([16, CH], BF16, tag="bscs", name="bscs", bufs=1)
            zp = sp.tile([16, 4], F32, tag="bszp", name="bszp", bufs=1)
            for p in range(4):
                nc.scalar.activation(out=cs[:, p * 256 : (p + 1) * 256], in_=q1[:, p * 256 : (p + 1) * 256],
                                     func=EXP, bias=qb16[bi][:], accum_out=zp[:, p : p + 1])
            nc.sync.dma_start(out=csd[bi], in_=cs[:])
            psz = pp.tile([128, 4], F32, tag="pstiny", bufs=1)
            nc.tensor.matmul(psz[0:1, :], ones16[:], zp[:], start=True, stop=True)
            zi = sp.tile([1, 4], F32, tag="zi", name="zi", bufs=1)
            nc.vector.reciprocal(zi[:], psz[0:1, :])
            psb = pp.tile([128, 4], F32, tag="pstiny", bufs=1)
            nc.tensor.matmul(psb[:], ones1r[:], zi[:], start=True, stop=True)
            nc.vector.tensor_copy(att[bi]["zinvb"][:], psb[:])

        def load_bcast(dram_row):
            t = sp.tile([128, CH], BF16, tag="bcast", name="bcast", bufs=4)
            nc.sync.dma_start(out=t[:], in_=bcast_row(dram_row, 128, CH))
            return t

        def load_z(dram, chunk, tag="zch"):
            zt = sp.tile([128, 3, CH], BF16, tag=tag, name=tag, bufs=2)
            for m in range(3):
                nc.sync.dma_start(out=zt[:, m, :], in_=dram[m, :, chunk * CH : (chunk + 1) * CH])
            return zt

        def normalize(zt, event, chunk, gps=False):
            rb = load_bcast(rd[event][0, chunk, :])
            mrb = load_bcast(rd[event][1, chunk, :])
            zn = sp.tile([128, 3, CH], BF16, tag="zn", name="zn", bufs=2)
            nc.vector.tensor_mul(zn[:], zt[:], rawap(rb[:], [[0, 3], [1, CH]]))
            eng = nc.gpsimd if gps else nc.vector
            eng.tensor_sub(zn[:], zn[:], rawap(mrb[:], [[0, 3], [1, CH]]))
            return zn

        # ================= S0: conv + pw1 + LN1_0 stats + q0 =================
        xslices = {}
        # zero the 3 rotating xps buffers once; interiors are overwritten by
        # each slice DMA, pad borders stay zero across rotations
        for _i in range(3):
            xz = p1.tile([128, 2, PSL], BF16, tag="xps", name="xps", bufs=3)
            nc.vector.memset(xz[:], 0.0)

        def load_slice(ts_):
            xs = p1.tile([128, 2, PSL], BF16, tag="xps", name="xps", bufs=3)
            for cti in range(2):
                dst = xs[:, cti, :].rearrange("p (h w) -> p h w", h=PWD)
                nc.sync.dma_start(out=dst[:, 1:33, 1:33], in_=xbf[cti, :, ts_, :].rearrange("p (h w) -> p h w", h=H))
            xslices[ts_] = xs

        for t in range(T):
            for ts_ in (t - 1, t, t + 1):
                if 0 <= ts_ < T and ts_ not in xslices:
                    load_slice(ts_)
            yact = p1.tile([128, 2, CH], BF16, tag="yact", name="yact")
            for cti in range(2):
                ok = lambda tp: 0 <= t + tp // 9 - 1 < T
                pe_here = [tp for tp in PE_TAPS if ok(tp)]
                dve_here = [tp for tp in DVE_TAPS if ok(tp)]
                gps_here = [tp for tp in GPS_TAPS if ok(tp)]

                def xsrc(tp):
                    dt_, dh, dw = tp // 9, (tp % 9) // 3, tp % 3
                    xv = xslices[t + dt_ - 1][:, cti, :].rearrange("p (h w) -> p h w", h=PWD)
                    return xv[:, dh : dh + 32, dw : dw + 32]

                ps_c = None
                if pe_here:
                    ps_c = pp.tile([128, CH], F32, tag="psA")
                    for half in range(2):
                        for j, tp in enumerate(pe_here):
                            dt_, dh, dw = tp // 9, (tp % 9) // 3, tp % 3
                            xv = xslices[t + dt_ - 1][:, cti, :].rearrange("p (h w) -> p h w", h=PWD)
                            nc.tensor.matmul(ps_c[:, half * 512 : (half + 1) * 512],
                                             diag_t[:, PE_TAPS.index(tp), cti, :],
                                             xv[:, dh + 16 * half : dh + 16 * half + 16, dw : dw + 32],
                                             start=(j == 0), stop=(j == len(pe_here) - 1))
                silu_src = ps_c[:] if ps_c is not None else None
                if dve_here or gps_here:
                    acc = p1.tile([128, CH], F32, tag="cacc", name="cacc", bufs=2)
                    accv = acc[:].rearrange("p (h w) -> p h w", h=H)
                    first = True
                    for tp in dve_here:
                        wcol = dwT[:, cti, tp : tp + 1]
                        if first:
                            nc.vector.tensor_scalar_mul(out=accv, in0=xsrc(tp), scalar1=wcol)
                        else:
                            nc.vector.scalar_tensor_tensor(out=accv, in0=xsrc(tp), scalar=wcol, in1=accv, op0=OP.mult, op1=OP.add)
                        first = False
                    gacc = None
                    for gj, tp in enumerate(gps_here):
                        wcol = dwT[:, cti, tp : tp + 1]
                        if gj == 0:
                            gacc = p1.tile([128, CH], F32, tag="gacc", name="gacc", bufs=1)
                            gaccv = gacc[:].rearrange("p (h w) -> p h w", h=H)
                            nc.gpsimd.tensor_scalar_mul(out=gaccv, in0=xsrc(tp), scalar1=wcol)
                        else:
                            nc.gpsimd.scalar_tensor_tensor(out=gaccv, in0=xsrc(tp), scalar=wcol, in1=gaccv, op0=OP.mult, op1=OP.add)
                    if gacc is not None and not first:
                        nc.vector.tensor_add(acc[:], acc[:], gacc[:])
                    elif gacc is not None:
                        acc = gacc
                    if ps_c is not None:
                        nc.vector.scalar_tensor_tensor(out=acc[:], in0=ps_c[:], scalar=1.0, in1=acc[:], op0=OP.mult, op1=OP.add)
                    silu_src = acc[:]
                nc.scalar.activation(out=yact[:, cti, :], in_=silu_src, func=SILU, bias=dwb_t[:, cti : cti + 1])
            zt = sp.tile([128, 3, CH], BF16, tag="zch", name="zch", bufs=2)
            for m in range(3):
                ps1 = pp.tile([128, CH], F32, tag="psA")
                for half in range(2):
                    hsl = slice(half * 512, (half + 1) * 512)
                    for kt in range(2):
                        nc.tensor.matmul(ps1[:, hsl], pw1_t[kt][:, m * 128 : (m + 1) * 128], yact[:, kt, hsl], start=(kt == 0), stop=(kt == 1))
                for ph_ in range(2):
                    nc.scalar.activation(out=nat2pm_out(zt[:, m, :])[:, ph_], in_=nat2pm_in(ps1[:])[:, ph_], func=AF.Identity, bias=pw1b_t[:, m : m + 1])
                nc.sync.dma_start(out=zst[0][m, :, t * CH : (t + 1) * CH], in_=zt[:, m, :])
            ln_stats(zt, bw[0]["wq"], 0, t, zsq_on_act=True)

        # ================= per-block sweeps =================
        def sweep_A(bi, zsrc, event):
            a = att[bi]
            for chunk in range(NCH):
                zt = load_z(zsrc, chunk)
                zn = normalize(zt, event, chunk)
                csb = load_bcast(csd[bi][chunk, :])
                vt = sp.tile([128, 3, CH], BF16, tag="vch", name="vch", bufs=2)
                for m in range(3):
                    psv = pp.tile([128, CH], F32, tag="psA")
                    for half in range(2):
                        hsl = slice(half * 512, (half + 1) * 512)
                        for kt in range(3):
                            nc.tensor.matmul(psv[:, hsl], bw[bi]["wv"][kt][:, m * 128 : (m + 1) * 128], zn[:, kt, hsl], start=(kt == 0), stop=(kt == 2))
                    nc.scalar.activation(out=vt[:, m, :], in_=psv[:], func=AF.Relu, bias=bw[bi]["vB"][:, m : m + 1])
                    nc.sync.dma_start(out=vd[bi][m, :, chunk * CH : (chunk + 1) * CH], in_=vt[:, m, :])
                junk = sp.tile([128, 3, CH], BF16, tag="junk", name="junk", bufs=1)
                csb3 = rawap(csb[:], [[0, 3], [1, CH]])
                nc.vector.tensor_mul(junk[:], zn[:], csb3)
                cvch = sp.tile([128, 3, 4], F32, tag="cvch", name="cvch", bufs=2)
                for m in range(3):
                    for p in range(4):
                        if (m + p) % 2 == 0:
                            nc.vector.tensor_reduce(cvch[:, m, p : p + 1], junk[:, m, p * 256 : (p + 1) * 256],
                                                    axis=mybir.AxisListType.X, op=OP.add)
                        else:
                            nc.scalar.activation(out=junk[:, m, p * 256 : (p + 1) * 256], in_=junk[:, m, p * 256 : (p + 1) * 256],
                                                 func=AF.Copy, accum_out=cvch[:, m, p : p + 1])
                nc.vector.tensor_add(a["cvacc"][:], a["cvacc"][:], cvch[:])

        def finalize_cv(bi):
            a = att[bi]
            cvb = sp.tile([128, 3, 4], BF16, tag="cvb", name="cvb", bufs=1)
            nc.vector.tensor_copy(cvb[:], a["cvacc"][:])
            cvf = sp.tile([128, 3, 4], F32, tag="cvf", name="cvf", bufs=1)
            for m in range(3):
                psc = pp.tile([128, 4], F32, tag="pstiny", bufs=1)
                for kt in range(3):
                    nc.tensor.matmul(psc[:], bw[bi]["wk"][kt][:, m * 128 : (m + 1) * 128], cvb[:, kt, :], start=(kt == 0), stop=(kt == 2))
                nc.vector.scalar_tensor_tensor(out=cvf[:, m, :], in0=psc[:], scalar=bw[bi]["kB"][:, m : m + 1], in1=a["zinvb"][:], op0=OP.add, op1=OP.mult)
            for kt in range(3):
                for p in range(4):
                    nc.vector.tensor_scalar_mul(out=a["wop"][kt][:, p, :], in0=bw[bi]["wo"][kt][:], scalar1=cvf[:, kt, p : p + 1])

        def sweep_B(bi, zsrc, zdst, event):
            a = att[bi]
            for chunk in range(NCH):
                zt = load_z(zsrc, chunk)
                vt = sp.tile([128, 3, CH], BF16, tag="vch", name="vch", bufs=2)
                for m in range(3):
                    nc.sync.dma_start(out=vt[:, m, :], in_=vd[bi][m, :, chunk * CH : (chunk + 1) * CH])
                zm = sp.tile([128, 3, CH], BF16, tag="zm", name="zm", bufs=2)
                for m in range(3):
                    pso = pp.tile([128, CH], F32, tag="psA")
                    for p in range(4):
                        sl = slice(p * 256, (p + 1) * 256)
                        for kt in range(3):
                            nc.tensor.matmul(pso[:, sl], a["wop"][kt][:, p, m * 128 : (m + 1) * 128], vt[:, kt, sl], start=(kt == 0), stop=False)
                        nc.tensor.matmul(pso[:, sl], eye_t[:], zt[:, m, sl], start=False, stop=True)
                    nc.scalar.activation(out=zm[:, m, :], in_=pso[:], func=AF.Identity, bias=bw[bi]["woB"][:, m : m + 1])
                    nc.sync.dma_start(out=zdst[m, :, chunk * CH : (chunk + 1) * CH], in_=zm[:, m, :])
                ln_stats(zm, None, event, chunk, zsq_on_act=False)

        def sweep_F(bi, zsrc, zdst, event_in, event_out, wq_next, last):
            for chunk in range(NCH):
                zt = load_z(zsrc, chunk)
                zn = normalize(zt, event_in, chunk)
                z2 = sp.tile([128, 3, CH], BF16, tag="z2", name="z2", bufs=2)
                ht = sp.tile([128, 6, CH], BF16, tag="ht", name="ht", bufs=1)
                for m6 in range(6):
                    ps1 = pp.tile([128, CH], F32, tag="psA")
                    for half in range(2):
                        hsl = slice(half * 512, (half + 1) * 512)
                        for kt in range(3):
                            nc.tensor.matmul(ps1[:, hsl], bw[bi]["ff1"][kt][:, m6 * 128 : (m6 + 1) * 128], zn[:, kt, hsl], start=(kt == 0), stop=(kt == 2))
                    nc.scalar.activation(out=ht[:, m6, :], in_=ps1[:], func=SILU, bias=bw[bi]["ff1B"][:, m6 : m6 + 1])
                for m in range(3):
                    psf = pp.tile([128, CH], F32, tag="psA")
                    for half in range(2):
                        hsl = slice(half * 512, (half + 1) * 512)
                        for m6 in range(6):
                            nc.tensor.matmul(psf[:, hsl], bw[bi]["ff2"][m6][:, m * 128 : (m + 1) * 128], ht[:, m6, hsl], start=(m6 == 0), stop=False)
                        nc.tensor.matmul(psf[:, hsl], eye_t[:], zt[:, m, hsl], start=False, stop=True)
                    nc.scalar.activation(out=z2[:, m, :], in_=psf[:], func=AF.Identity, bias=bw[bi]["ff2B"][:, m : m + 1])
                    if zdst is not None:
                        nc.sync.dma_start(out=zdst[m, :, chunk * CH : (chunk + 1) * CH], in_=z2[:, m, :])
                if not last:
                    ln_stats(z2, wq_next, event_out, chunk, zsq_on_act=False)
                else:
                    for m in range(2):
                        ot = sp.tile([128, CH], F32, tag="ot", name="ot", bufs=1)
                        ps2 = pp.tile([128, CH], F32, tag="psA")
                        for half in range(2):
                            hsl = slice(half * 512, (half + 1) * 512)
                            for kt in range(3):
                                nc.tensor.matmul(ps2[:, hsl], pw2_t[kt][:, m * 128 : (m + 1) * 128], z2[:, kt, hsl], start=(kt == 0), stop=(kt == 2))
                        for ph_ in range(2):
                            nc.scalar.activation(out=pm2nat_out(ot[:])[:, ph_], in_=pm2nat_in(ps2[:])[:, ph_], func=AF.Identity, bias=pw2b_t[:, m : m + 1])
                        nc.sync.dma_start(out=out[m * 128 : (m + 1) * 128, chunk * CH : (chunk + 1) * CH], in_=ot[:])

        if STAGE >= 2:
            batch_math(0, bi=0)
            if SUB >= 2:
                sweep_A(0, zst[0], 0)
            if SUB >= 3:
                finalize_cv(0)
            if SUB >= 4:
                sweep_B(0, zst[0], zst[1], 1)
            if SUB >= 5:
                batch_math(1)
                sweep_F(0, zst[1], zst[2], 1, 2, bw[1]["wq"], last=False)
        if STAGE >= 3:
            batch_math(2, bi=1)
            sweep_A(1, zst[2], 2)
            finalize_cv(1)
            sweep_B(1, zst[2], zst[3], 3)
            batch_math(3)
            sweep_F(1, zst[3], None, 3, None, None, last=True)

    nc.compile()
    return nc


_NC = None


def _get_nc():
    global _NC
    if _NC is None:
        _NC = build()
    return _NC


def _prep(inputs):
    f32 = lambda a: np.ascontiguousarray(np.asarray(a, np.float32))
    bf = lambda a: np.ascontiguousarray(np.asarray(a, np.float32)).astype(NPBF16)
    dw = f32(inputs["dw_w"]).reshape(C, 27)
    base = {
        "dwTap": np.ascontiguousarray(dw.reshape(2, 128, 27)),
        "dwB": np.ascontiguousarray(f32(inputs["dw_b"]).reshape(2, 128).T),
        "eye": np.eye(128, dtype=np.float32).astype(NPBF16),
        "pw1W": bf(inputs["pw1_w"]),
        "pw1B": np.ascontiguousarray(f32(inputs["pw1_b"]).reshape(3, 128).T),
        "pw2W": bf(inputs["pw2_w"]),
        "pw2B": np.ascontiguousarray(f32(inputs["pw2_b"]).reshape(2, 128).T),
    }
    diag = np.zeros((max(1, len(PE_TAPS)), 2, 128, 128), np.float32)
    for ti, tp in enumerate(PE_TAPS):
        for cti in range(2):
            np.fill_diagonal(diag[ti, cti], dw[cti * 128 : (cti + 1) * 128, tp])
    base["dwDiag"] = diag.astype(NPBF16)
    qf = np.zeros((NBLK, 1), np.float32)
    for i in range(NBLK):
        qkvW = f32(inputs["ln1_g"][i])[:, None] * f32(inputs["qkv_w"][i])
        qkvB = f32(inputs["ln1_b"][i]) @ f32(inputs["qkv_w"][i]) + f32(inputs["qkv_b"][i])
        ff1W = f32(inputs["ln2_g"][i])[:, None] * f32(inputs["ff1_w"][i])
        ff1B = f32(inputs["ln2_b"][i]) @ f32(inputs["ff1_w"][i]) + f32(inputs["ff1_b"][i])
        wqb = np.ascontiguousarray(qkvW[:, 0:1]).astype(NPBF16)
        qf[i, 0] = -float(np.asarray(wqb, np.float32).sum())
        base.update({
            f"wq{i}": wqb,
            f"wk{i}": np.ascontiguousarray(qkvW[:, 1 : 1 + D]).astype(NPBF16),
            f"wv{i}": np.ascontiguousarray(qkvW[:, 1 + D :]).astype(NPBF16),
            f"qB{i}": np.ascontiguousarray(qkvB[0:1].reshape(1, 1)),
            f"kB{i}": np.ascontiguousarray(qkvB[1 : 1 + D].reshape(3, 128).T),
            f"vB{i}": np.ascontiguousarray(qkvB[1 + D :].reshape(3, 128).T),
            f"woW{i}": bf(inputs["wo_w"][i]),
            f"woB{i}": np.ascontiguousarray(f32(inputs["wo_b"][i]).reshape(3, 128).T),
            f"ff1W{i}": ff1W.astype(NPBF16),
            f"ff1B{i}": np.ascontiguousarray(ff1B.reshape(6, 128).T),
            f"ff2W{i}": bf(inputs["ff2_w"][i]),
            f"ff2B{i}": np.ascontiguousarray(f32(inputs["ff2_b"][i]).reshape(3, 128).T),
        })
    base["qfix"] = qf
    return base


def kernel(**inputs):
    base = _prep(inputs)
    x = np.asarray(inputs["x"], np.float32)
    in_maps = []
    for b in range(B):
        xb = np.ascontiguousarray(x[b].reshape(2, 128, T, H * W)).astype(NPBF16)
        in_maps.append(dict(base, xbf=xb))
    nc = _get_nc()
    trace = bool(int(os.environ.get("KERNEL_TRACE", "0")))
    res = run_bass_kernel_spmd(nc, in_maps, list(range(B)), trace=trace)
    kernel.last_exec_ns = res.exec_time_ns
    kernel.last_profile = res.profile_json
    kernel.last_results = res.results
    outs = [res.results[b]["out"].reshape(OUTC, T, H, W) for b in range(B)]
    return np.stack(outs).astype(np.float32)


# revision 11
# speedup vs baseline: 1.0693x; 1.0235x over previous
"""MobileViTV2 block kernel v2 — 8 TRN2 cores, data-parallel over batch.

All-bf16 datapath (fp32 PSUM), CH=1024 t-slice chunks, patch-major token
order j = p*256 + hp*16 + wp inside each chunk. z streams through DRAM in
bf16 between sweeps. Per block:
  sweep_A: normalize -> v GEMM -> relu -> spill v; zsum += cs*zn (ttr)
     [k-GEMM eliminated: cv = Wk^T zsum * (1/Z) + kB]
  sweep_B: per-patch wo GEMMs with cv FOLDED INTO wo weights; residual
     added via identity-matmul into PSUM; LN2 stats.
  sweep_F: FFN (+residual via identity-matmul) + next LN1 stats + q GEMM;
     last block variant does pw2 -> out (natural order restored in the
     PSUM->SBUF move).
LN stat math is batched: per-chunk stat rows (sum/sumsq/qraw via ones/wq
matmuls) bounce through DRAM into [16,1024] chunk-per-partition tiles so
the stat pipeline runs once per LN event, not once per chunk. R and M*R
rows bounce back via DMA broadcast reads. Conv taps split across PE
(diag-matmul), DVE and GPSIMD (env-tunable).
"""

import sys

sys.path.insert(0, "/opt/trn_rl_repo")
import os
import numpy as np
from contextlib import ExitStack

import concourse.bass as bass
import concourse.mybir as mybir
import concourse.tile as tile
from concourse import bacc
from concourse.bass_utils import run_bass_kernel_spmd

F32 = mybir.dt.float32
F32R = mybir.dt.float32r
BF16 = mybir.dt.bfloat16
AF = mybir.ActivationFunctionType
OP = mybir.AluOpType
NPBF16 = mybir.dt.np(BF16)

B, C, T, H, W = 8, 256, 16, 32, 32
D, OUTC, NBLK, FF = 384, 256, 2, 768
NTOK = T * H * W
CH = 1024
NCH = 16
PWD = 34
PSL = PWD * PWD
EPS = 1e-5

STAGE = int(os.environ.get("KERNEL_STAGE", "3"))
SUB = int(os.environ.get("KERNEL_SUB", "9"))
SIM_SAFE = bool(int(os.environ.get("KERNEL_SIM_SAFE", "0")))
N_PE_TAPS = int(os.environ.get("KERNEL_PE_TAPS", "27"))
N_GPS_TAPS = int(os.environ.get("KERNEL_GPS_TAPS", "0"))
ALL_TAPS = list(range(27))
PE_TAPS = ALL_TAPS[:N_PE_TAPS]
GPS_TAPS = ALL_TAPS[N_PE_TAPS : N_PE_TAPS + N_GPS_TAPS]
DVE_TAPS = ALL_TAPS[N_PE_TAPS + N_GPS_TAPS :]

SILU = AF.Square if SIM_SAFE else AF.Silu
EXP = AF.Square if SIM_SAFE else AF.Exp


def rawap(base, dims):
    return bass.AP(tensor=base.tensor, offset=base.offset, ap=[base.ap[0]] + dims)


def bcast_row(row, n, cols):
    """DRAM row -> [n, cols] broadcast-read AP."""
    return bass.AP(tensor=row.tensor, offset=row.offset, ap=[[0, n], [1, cols]])


# patch-major <-> natural permute views (within one 1024-token t-slice)
def nat2pm_in(ps):
    return ps.rearrange("p (hp ph wp pw) -> p ph pw hp wp", hp=16, ph=2, wp=16, pw=2)


def nat2pm_out(zslice):
    return zslice.rearrange("p (ph pw hp wp) -> p ph pw hp wp", ph=2, pw=2, hp=16, wp=16)


def pm2nat_in(ps):
    return ps.rearrange("p (ph pw hp wp) -> p ph hp wp pw", ph=2, pw=2, hp=16, wp=16)


def pm2nat_out(oslice):
    return oslice.rearrange("p (hp ph wp pw) -> p ph hp wp pw", hp=16, ph=2, wp=16, pw=2)


def build():
    nc = bacc.Bacc("TRN2", target_bir_lowering=False, debug=False, num_devices=8)

    xbf = nc.dram_tensor("xbf", [2, 128, T, H * W], BF16, kind="ExternalInput").ap()
    dwTap = nc.dram_tensor("dwTap", [2, 128, 27], F32, kind="ExternalInput").ap()
    dwBd = nc.dram_tensor("dwB", [128, 2], F32, kind="ExternalInput").ap()
    dwDiag = nc.dram_tensor("dwDiag", [max(1, len(PE_TAPS)), 2, 128, 128], BF16, kind="ExternalInput").ap()
    eyeD = nc.dram_tensor("eye", [128, 128], BF16, kind="ExternalInput").ap()
    pw1W = nc.dram_tensor("pw1W", [C, D], BF16, kind="ExternalInput").ap()
    pw1B = nc.dram_tensor("pw1B", [128, 3], F32, kind="ExternalInput").ap()
    pw2W = nc.dram_tensor("pw2W", [D, OUTC], BF16, kind="ExternalInput").ap()
    pw2B = nc.dram_tensor("pw2B", [128, 2], F32, kind="ExternalInput").ap()
    blk = []
    for i in range(NBLK):
        blk.append(dict(
            wq=nc.dram_tensor(f"wq{i}", [D, 1], BF16, kind="ExternalInput").ap(),
            wk=nc.dram_tensor(f"wk{i}", [D, D], BF16, kind="ExternalInput").ap(),
            wv=nc.dram_tensor(f"wv{i}", [D, D], BF16, kind="ExternalInput").ap(),
            qB=nc.dram_tensor(f"qB{i}", [1, 1], F32, kind="ExternalInput").ap(),
            kB=nc.dram_tensor(f"kB{i}", [128, 3], F32, kind="ExternalInput").ap(),
            vB=nc.dram_tensor(f"vB{i}", [128, 3], F32, kind="ExternalInput").ap(),
            woW=nc.dram_tensor(f"woW{i}", [D, D], BF16, kind="ExternalInput").ap(),
            woB=nc.dram_tensor(f"woB{i}", [128, 3], F32, kind="ExternalInput").ap(),
            ff1W=nc.dram_tensor(f"ff1W{i}", [D, FF], BF16, kind="ExternalInput").ap(),
            ff1B=nc.dram_tensor(f"ff1B{i}", [128, 6], F32, kind="ExternalInput").ap(),
            ff2W=nc.dram_tensor(f"ff2W{i}", [FF, D], BF16, kind="ExternalInput").ap(),
            ff2B=nc.dram_tensor(f"ff2B{i}", [128, 3], F32, kind="ExternalInput").ap(),
        ))
    qfix = nc.dram_tensor("qfix", [NBLK, 1], F32, kind="ExternalInput").ap()

    out = nc.dram_tensor("out", [OUTC, NTOK], F32, kind="ExternalOutput").ap()
    zst = [nc.dram_tensor(n, [3, 128, NTOK], BF16, kind="ExternalOutput").ap()
           for n in ("z0", "zm0", "z1", "zm1")]
    vd = [nc.dram_tensor(f"v{i}", [3, 128, NTOK], BF16).ap() for i in range(NBLK)]
    statd = [nc.dram_tensor(f"statd{e}", [3, NCH, CH], BF16).ap() for e in range(4)]
    rd = [nc.dram_tensor(f"rd{e}", [2, NCH, CH], BF16).ap() for e in range(4)]
    csd = [nc.dram_tensor(f"csd{i}", [NCH, CH], BF16).ap() for i in range(NBLK)]

    with ExitStack() as ctx:
        tc = ctx.enter_context(tile.TileContext(nc))
        wpool = ctx.enter_context(tc.tile_pool(name="w", bufs=1))
        sp = ctx.enter_context(tc.tile_pool(name="s", bufs=2))
        pp = ctx.enter_context(tc.tile_pool(name="ps", bufs=3, space="PSUM"))
        cvp = ctx.enter_context(tc.tile_pool(name="cv", bufs=1))
        p1 = ctx.enter_context(tc.tile_pool(name="p1", bufs=2))

        def wt(name, dram, kdim, mdim, dt=BF16):
            tiles = []
            for ki in range((kdim + 127) // 128):
                t = wpool.tile([128, mdim], dt, tag=f"{name}{ki}")
                nc.sync.dma_start(out=t[:], in_=dram[ki * 128 : (ki + 1) * 128, :])
                tiles.append(t)
            return tiles

        def ftile(name, dram, cols):
            t = wpool.tile([128, cols], F32, tag=name)
            nc.sync.dma_start(out=t[:], in_=dram)
            return t

        dwT = wpool.tile([128, 2, 27], F32, tag="dwT")
        for cti in range(2):
            nc.sync.dma_start(out=dwT[:, cti, :], in_=dwTap[cti])
        dwb_t = ftile("dwB", dwBd, 2)
        diag_t = None
        if PE_TAPS:
            diag_t = wpool.tile([128, len(PE_TAPS), 2, 128], BF16, tag="diag")
            for ti in range(len(PE_TAPS)):
                for cti in range(2):
                    nc.sync.dma_start(out=diag_t[:, ti, cti, :], in_=dwDiag[ti, cti])
        eye_t = wpool.tile([128, 128], BF16, tag="eye")
        nc.sync.dma_start(out=eye_t[:], in_=eyeD)
        pw1_t = wt("pw1", pw1W, C, D)
        pw1b_t = ftile("pw1B", pw1B, 3)
        pw2_t = wt("pw2", pw2W, D, OUTC)
        pw2b_t = ftile("pw2B", pw2B, 2)
        bw = []
        for i in range(NBLK):
            bw.append(dict(
                wq=wt(f"wq{i}_", blk[i]["wq"], D, 1),
                wk=wt(f"wk{i}_", blk[i]["wk"], D, D),
                wv=wt(f"wv{i}_", blk[i]["wv"], D, D),
                kB=ftile(f"kB{i}", blk[i]["kB"], 3),
                vB=ftile(f"vB{i}", blk[i]["vB"], 3),
                wo=wt(f"wo{i}_", blk[i]["woW"], D, D),
                woB=ftile(f"woB{i}", blk[i]["woB"], 3),
                ff1=wt(f"ff1{i}_", blk[i]["ff1W"], D, FF),
                ff1B=ftile(f"ff1B{i}", blk[i]["ff1B"], 6),
                ff2=wt(f"ff2{i}_", blk[i]["ff2W"], FF, D),
                ff2B=ftile(f"ff2B{i}", blk[i]["ff2B"], 3),
            ))
        ones_bf = wpool.tile([128, 1], BF16, tag="ones_bf")
        nc.vector.memset(ones_bf[:], 1.0)
        ones16 = wpool.tile([16, 1], F32, tag="ones16")
        nc.vector.memset(ones16[:], 1.0)
        ones1r = wpool.tile([1, 128], F32, tag="ones1r")
        nc.vector.memset(ones1r[:], 1.0)
        eps16 = wpool.tile([16, 1], F32, tag="eps16")
        nc.vector.memset(eps16[:], EPS)
        qb16 = [wpool.tile([16, 1], F32, tag=f"qb16_{i}", name=f"qb16_{i}") for i in range(NBLK)]
        sq16 = [wpool.tile([16, 1], F32, tag=f"sq16_{i}", name=f"sq16_{i}") for i in range(NBLK)]
        for i in range(NBLK):
            nc.sync.dma_start(out=qb16[i][:], in_=bcast_row(blk[i]["qB"][0, :], 16, 1))
            nc.sync.dma_start(out=sq16[i][:], in_=bcast_row(qfix[i, :], 16, 1))

        wop = [cvp.tile([128, 4, 384], BF16, tag=f"wop{k}", name=f"wop{k}") for k in range(3)]
        att = []
        for i in range(NBLK):
            a = dict(
                cvacc=cvp.tile([128, 3, 4], F32, tag=f"cvacc{i}", name=f"cvacc{i}"),
                zinvb=cvp.tile([128, 4], F32, tag=f"zinvb{i}", name=f"zinvb{i}"),
                wop=wop,
            )
            nc.vector.memset(a["cvacc"][:], 0.0)
            att.append(a)

        # ---------- helpers ----------
        def ln_stats(zt, wq_tiles, event, chunk, zsq_on_act):
            zsq = sp.tile([128, 3, CH], BF16, tag="zsq", name="zsq", bufs=1)
            if zsq_on_act:
                nc.scalar.activation(out=zsq[:], in_=zt[:], func=AF.Square)
            else:
                nc.vector.tensor_mul(zsq[:], zt[:], zt[:])
            ps = pp.tile([128, CH], F32, tag="pstat", bufs=1)
            for half in range(2):
                hsl = slice(half * 512, (half + 1) * 512)
                for kt in range(3):
                    nc.tensor.matmul(ps[0:1, hsl], ones_bf[:], zt[:, kt, hsl], start=(kt == 0), stop=(kt == 2))
                    nc.tensor.matmul(ps[32:33, hsl], ones_bf[:], zsq[:, kt, hsl], start=(kt == 0), stop=(kt == 2))
                    if wq_tiles is not None:
                        nc.tensor.matmul(ps[64:65, hsl], wq_tiles[kt][:], zt[:, kt, hsl], start=(kt == 0), stop=(kt == 2))
            sb = sp.tile([65, CH], BF16, tag="sbstat", name="sbstat", bufs=2)
            nc.vector.tensor_copy(sb[:], ps[0:65, :])
            nc.sync.dma_start(out=statd[event][0, chunk, :], in_=sb[0:1, :])
            nc.sync.dma_start(out=statd[event][1, chunk, :], in_=sb[32:33, :])
            if wq_tiles is not None:
                nc.sync.dma_start(out=statd[event][2, chunk, :], in_=sb[64:65, :])

        def batch_math(event, bi=None):
            with nc.allow_low_precision(reason="LN stat math in bf16 is within error budget"):
                return _batch_math(event, bi)

        def _batch_math(event, bi=None):
            bs = sp.tile([16, 3, CH], BF16, tag="bs", name="bs", bufs=1)
            for s in range(3 if bi is not None else 2):
                nc.sync.dma_start(out=bs[:, s, :], in_=statd[event][s])
            tmp = sp.tile([16, 5, CH], BF16, tag="bstmp", name="bstmp", bufs=1)
            M, t2, R, MR, q1 = (tmp[:, j, :] for j in range(5))
            nc.vector.tensor_scalar_mul(out=M, in0=bs[:, 0, :], scalar1=1.0 / D)
            nc.vector.tensor_mul(t2, M, M)
            nc.vector.scalar_tensor_tensor(out=t2, in0=bs[:, 1, :], scalar=1.0 / D, in1=t2, op0=OP.mult, op1=OP.subtract)
            nc.scalar.activation(out=t2, in_=t2, func=AF.Sqrt, bias=eps16[:])
            nc.vector.reciprocal(R, t2)
            nc.vector.tensor_mul(MR, M, R)
            rb = sp.tile([16, 2, CH], BF16, tag="rbf", name="rbf", bufs=1)
            nc.vector.tensor_copy(rb[:, 0, :], R)
            nc.vector.tensor_copy(rb[:, 1, :], MR)
            nc.sync.dma_start(out=rd[event][0], in_=rb[:, 0, :])
            nc.sync.dma_start(out=rd[event][1], in_=rb[:, 1, :])
            if bi is None:
                return
            nc.vector.scalar_tensor_tensor(out=q1, in0=M, scalar=sq16[bi][:], in1=bs[:, 2, :], op0=OP.mult, op1=OP.add)
            nc.vector.tensor_mul(q1, q1, R)
            cs = sp.tile([16, CH], BF16, tag="bscs", name="bscs", bufs=1)
            zp = sp.tile([16, 4], F32, tag="bszp", name="bszp", bufs=1)
            for p in range(4):
                nc.scalar.activation(out=cs[:, p * 256 : (p + 1) * 256], in_=q1[:, p * 256 : (p + 1) * 256],
                                     func=EXP, bias=qb16[bi][:], accum_out=zp[:, p : p + 1])
            nc.sync.dma_start(out=csd[bi], in_=cs[:])
            psz = pp.tile([128, CH], F32, tag="pstat", bufs=1)
            nc.tensor.matmul(psz[0:1, 0:4], ones16[:], zp[:], start=True, stop=True)
            zi = sp.tile([1, 4], F32, tag="zi", name="zi", bufs=1)
            nc.vector.reciprocal(zi[:], psz[0:1, 0:4])
            psb = pp.tile([128, CH], F32, tag="pstat", bufs=1)
            nc.tensor.matmul(psb[:, 0:4], ones1r[:], zi[:], start=True, stop=True)
            nc.vector.tensor_copy(att[bi]["zinvb"][:], psb[:, 0:4])

        def load_bcast(dram_row):
            t = sp.tile([128, CH], BF16, tag="bcast", name="bcast", bufs=4)
            nc.sync.dma_start(out=t[:], in_=bcast_row(dram_row, 128, CH))
            return t

        def load_z(dram, chunk, tag="zch"):
            zt = sp.tile([128, 3, CH], BF16, tag=tag, name=tag, bufs=2)
            for m in range(3):
                nc.sync.dma_start(out=zt[:, m, :], in_=dram[m, :, chunk * CH : (chunk + 1) * CH])
            return zt

        def normalize(zt, event, chunk, gps=False):
            rb = load_bcast(rd[event][0, chunk, :])
            mrb = load_bcast(rd[event][1, chunk, :])
            zn = sp.tile([128, 3, CH], BF16, tag="zn", name="zn", bufs=2)
            nc.vector.tensor_mul(zn[:], zt[:], rawap(rb[:], [[0, 3], [1, CH]]))
            eng = nc.gpsimd if gps else nc.vector
            eng.tensor_sub(zn[:], zn[:], rawap(mrb[:], [[0, 3], [1, CH]]))
            return zn

        # ================= S0: conv + pw1 + LN1_0 stats + q0 =================
        xslices = {}
        # zero the 3 rotating xps buffers once; interiors are overwritten by
        # each slice DMA, pad borders stay zero across rotations
        for _i in range(3):
            xz = p1.tile([128, 2, PSL], BF16, tag="xps", name="xps", bufs=3)
            nc.vector.memset(xz[:], 0.0)

        def load_slice(ts_):
            xs = p1.tile([128, 2, PSL], BF16, tag="xps", name="xps", bufs=3)
            for cti in range(2):
                dst = xs[:, cti, :].rearrange("p (h w) -> p h w", h=PWD)
                nc.sync.dma_start(out=dst[:, 1:33, 1:33], in_=xbf[cti, :, ts_, :].rearrange("p (h w) -> p h w", h=H))
            xslices[ts_] = xs

        for t in range(T):
            for ts_ in (t - 1, t, t + 1):
                if 0 <= ts_ < T and ts_ not in xslices:
                    load_slice(ts_)
            yact = p1.tile([128, 2, CH], BF16, tag="yact", name="yact")
            for cti in range(2):
                ok = lambda tp: 0 <= t + tp // 9 - 1 < T
                pe_here = [tp for tp in PE_TAPS if ok(tp)]
                dve_here = [tp for tp in DVE_TAPS if ok(tp)]
                gps_here = [tp for tp in GPS_TAPS if ok(tp)]

                def xsrc(tp):
                    dt_, dh, dw = tp // 9, (tp % 9) // 3, tp % 3
                    xv = xslices[t + dt_ - 1][:, cti, :].rearrange("p (h w) -> p h w", h=PWD)
                    return xv[:, dh : dh + 32, dw : dw + 32]

                ps_c = None
                if pe_here:
                    ps_c = pp.tile([128, CH], F32, tag="psA")
                    for half in range(2):
                        for j, tp in enumerate(pe_here):
                            dt_, dh, dw = tp // 9, (tp % 9) // 3, tp % 3
                            xv = xslices[t + dt_ - 1][:, cti, :].rearrange("p (h w) -> p h w", h=PWD)
                            nc.tensor.matmul(ps_c[:, half * 512 : (half + 1) * 512],
                                             diag_t[:, PE_TAPS.index(tp), cti, :],
                                             xv[:, dh + 16 * half : dh + 16 * half + 16, dw : dw + 32],
                                             start=(j == 0), stop=(j == len(pe_here) - 1))
                silu_src = ps_c[:] if ps_c is not None else None
                if dve_here or gps_here:
                    acc = p1.tile([128, CH], F32, tag="cacc", name="cacc", bufs=2)
                    accv = acc[:].rearrange("p (h w) -> p h w", h=H)
                    first = True
                    for tp in dve_here:
                        wcol = dwT[:, cti, tp : tp + 1]
                        if first:
                            nc.vector.tensor_scalar_mul(out=accv, in0=xsrc(tp), scalar1=wcol)
                        else:
                            nc.vector.scalar_tensor_tensor(out=accv, in0=xsrc(tp), scalar=wcol, in1=accv, op0=OP.mult, op1=OP.add)
                        first = False
                    gacc = None
                    for gj, tp in enumerate(gps_here):
                        wcol = dwT[:, cti, tp : tp + 1]
                        if gj == 0:
                            gacc = p1.tile([128, CH], F32, tag="gacc", name="gacc", bufs=1)
                            gaccv = gacc[:].rearrange("p (h w) -> p h w", h=H)
                            nc.gpsimd.tensor_scalar_mul(out=gaccv, in0=xsrc(tp), scalar1=wcol)
                        else:
                            nc.gpsimd.scalar_tensor_tensor(out=gaccv, in0=xsrc(tp), scalar=wcol, in1=gaccv, op0=OP.mult, op1=OP.add)
                    if gacc is not None and not first:
                        nc.vector.tensor_add(acc[:], acc[:], gacc[:])
                    elif gacc is not None:
                        acc = gacc
                    if ps_c is not None:
                        nc.vector.scalar_tensor_tensor(out=acc[:], in0=ps_c[:], scalar=1.0, in1=acc[:], op0=OP.mult, op1=OP.add)
                    silu_src = acc[:]
                nc.scalar.activation(out=yact[:, cti, :], in_=silu_src, func=SILU, bias=dwb_t[:, cti : cti + 1])
            zt = sp.tile([128, 3, CH], BF16, tag="zch", name="zch", bufs=2)
            for m in range(3):
                ps1 = pp.tile([128, CH], F32, tag="psA")
                for half in range(2):
                    hsl = slice(half * 512, (half + 1) * 512)
                    for kt in range(2):
                        nc.tensor.matmul(ps1[:, hsl], pw1_t[kt][:, m * 128 : (m + 1) * 128], yact[:, kt, hsl], start=(kt == 0), stop=(kt == 1))
                for ph_ in range(2):
                    nc.scalar.activation(out=nat2pm_out(zt[:, m, :])[:, ph_], in_=nat2pm_in(ps1[:])[:, ph_], func=AF.Identity, bias=pw1b_t[:, m : m + 1])
                nc.sync.dma_start(out=zst[0][m, :, t * CH : (t + 1) * CH], in_=zt[:, m, :])
            ln_stats(zt, bw[0]["wq"], 0, t, zsq_on_act=True)

        # ================= per-block sweeps =================
        def sweep_A(bi, zsrc, event):
            a = att[bi]
            for chunk in range(NCH):
                zt = load_z(zsrc, chunk)
                zn = normalize(zt, event, chunk)
                csb = load_bcast(csd[bi][chunk, :])
                vt = sp.tile([128, 3, CH], BF16, tag="vch", name="vch", bufs=2)
                for m in range(3):
                    psv = pp.tile([128, CH], F32, tag="psA")
                    for half in range(2):
                        hsl = slice(half * 512, (half + 1) * 512)
                        for kt in range(3):
                            nc.tensor.matmul(psv[:, hsl], bw[bi]["wv"][kt][:, m * 128 : (m + 1) * 128], zn[:, kt, hsl], start=(kt == 0), stop=(kt == 2))
                    nc.scalar.activation(out=vt[:, m, :], in_=psv[:], func=AF.Relu, bias=bw[bi]["vB"][:, m : m + 1])
                    nc.sync.dma_start(out=vd[bi][m, :, chunk * CH : (chunk + 1) * CH], in_=vt[:, m, :])
                junk = sp.tile([128, 3, CH], BF16, tag="junk", name="junk", bufs=1)
                csb3 = rawap(csb[:], [[0, 3], [1, CH]])
                nc.vector.tensor_mul(junk[:], zn[:], csb3)
                cvch = sp.tile([128, 3, 4], F32, tag="cvch", name="cvch", bufs=2)
                for m in range(3):
                    for p in range(4):
                        if (m + p) % 2 == 0:
                            nc.vector.tensor_reduce(cvch[:, m, p : p + 1], junk[:, m, p * 256 : (p + 1) * 256],
                                                    axis=mybir.AxisListType.X, op=OP.add)
                        else:
                            nc.scalar.activation(out=junk[:, m, p * 256 : (p + 1) * 256], in_=junk[:, m, p * 256 : (p + 1) * 256],
                                                 func=AF.Copy, accum_out=cvch[:, m, p : p + 1])
                nc.vector.tensor_add(a["cvacc"][:], a["cvacc"][:], cvch[:])

        def finalize_cv(bi):
            a = att[bi]
            cvb = sp.tile([128, 3, 4], BF16, tag="cvb", name="cvb", bufs=1)
            nc.vector.tensor_copy(cvb[:], a["cvacc"][:])
            cvf = sp.tile([128, 3, 4], F32, tag="cvf", name="cvf", bufs=1)
            for m in range(3):
                psc = pp.tile([128, CH], F32, tag="pstat", bufs=1)
                for kt in range(3):
                    nc.tensor.matmul(psc[:, 0:4], bw[bi]["wk"][kt][:, m * 128 : (m + 1) * 128], cvb[:, kt, :], start=(kt == 0), stop=(kt == 2))
                nc.vector.scalar_tensor_tensor(out=cvf[:, m, :], in0=psc[:, 0:4], scalar=bw[bi]["kB"][:, m : m + 1], in1=a["zinvb"][:], op0=OP.add, op1=OP.mult)
            for kt in range(3):
                for p in range(4):
                    nc.vector.tensor_scalar_mul(out=a["wop"][kt][:, p, :], in0=bw[bi]["wo"][kt][:], scalar1=cvf[:, kt, p : p + 1])

        def sweep_B(bi, zsrc, zdst, event):
            a = att[bi]
            for chunk in range(NCH):
                zt = load_z(zsrc, chunk)
                vt = sp.tile([128, 3, CH], BF16, tag="vch", name="vch", bufs=2)
                for m in range(3):
                    nc.sync.dma_start(out=vt[:, m, :], in_=vd[bi][m, :, chunk * CH : (chunk + 1) * CH])
                zm = sp.tile([128, 3, CH], BF16, tag="zm", name="zm", bufs=2)
                for m in range(3):
                    pso = pp.tile([128, CH], F32, tag="psA")
                    for p in range(4):
                        sl = slice(p * 256, (p + 1) * 256)
                        for kt in range(3):
                            nc.tensor.matmul(pso[:, sl], a["wop"][kt][:, p, m * 128 : (m + 1) * 128], vt[:, kt, sl], start=(kt == 0), stop=False)
                        nc.tensor.matmul(pso[:, sl], eye_t[:], zt[:, m, sl], start=False, stop=True)
                    nc.scalar.activation(out=zm[:, m, :], in_=pso[:], func=AF.Identity, bias=bw[bi]["woB"][:, m : m + 1])
                    nc.sync.dma_start(out=zdst[m, :, chunk * CH : (chunk + 1) * CH], in_=zm[:, m, :])
                ln_stats(zm, None, event, chunk, zsq_on_act=False)

        def sweep_F(bi, zsrc, zdst, event_in, event_out, wq_next, last):
            for chunk in range(NCH):
                zt = load_z(zsrc, chunk)
                zn = normalize(zt, event_in, chunk)
                z2 = sp.tile([128, 3, CH], BF16, tag="z2", name="z2", bufs=2)
                ht = sp.tile([128, 6, CH], BF16, tag="ht", name="ht", bufs=1)
                for m6 in range(6):
                    ps1 = pp.tile([128, CH], F32, tag="psA")
                    for half in range(2):
                        hsl = slice(half * 512, (half + 1) * 512)
                        for kt in range(3):
                            nc.tensor.matmul(ps1[:, hsl], bw[bi]["ff1"][kt][:, m6 * 128 : (m6 + 1) * 128], zn[:, kt, hsl], start=(kt == 0), stop=(kt == 2))
                    nc.scalar.activation(out=ht[:, m6, :], in_=ps1[:], func=SILU, bias=bw[bi]["ff1B"][:, m6 : m6 + 1])
                for m in range(3):
                    psf = pp.tile([128, CH], F32, tag="psA")
                    for half in range(2):
                        hsl = slice(half * 512, (half + 1) * 512)
                        for m6 in range(6):
                            nc.tensor.matmul(psf[:, hsl], bw[bi]["ff2"][m6][:, m * 128 : (m + 1) * 128], ht[:, m6, hsl], start=(m6 == 0), stop=False)
                        nc.tensor.matmul(psf[:, hsl], eye_t[:], zt[:, m, hsl], start=False, stop=True)
                    nc.scalar.activation(out=z2[:, m, :], in_=psf[:], func=AF.Identity, bias=bw[bi]["ff2B"][:, m : m + 1])
                    if zdst is not None:
                        nc.sync.dma_start(out=zdst[m, :, chunk * CH : (chunk + 1) * CH], in_=z2[:, m, :])
                if not last:
                    ln_stats(z2, wq_next, event_out, chunk, zsq_on_act=False)
                else:
                    for m in range(2):
                        ot = sp.tile([128, CH], F32, tag="ot", name="ot", bufs=1)
                        ps2 = pp.tile([128, CH], F32, tag="pstat", bufs=1)
                        for half in range(2):
                            hsl = slice(half * 512, (half + 1) * 512)
                            for kt in range(3):
                                nc.tensor.matmul(ps2[:, hsl], pw2_t[kt][:, m * 128 : (m + 1) * 128], z2[:, kt, hsl], start=(kt == 0), stop=(kt == 2))
                        for ph_ in range(2):
                            nc.scalar.activation(out=pm2nat_out(ot[:])[:, ph_], in_=pm2nat_in(ps2[:])[:, ph_], func=AF.Identity, bias=pw2b_t[:, m : m + 1])
                        nc.sync.dma_start(out=out[m * 128 : (m + 1) * 128, chunk * CH : (chunk + 1) * CH], in_=ot[:])

        if STAGE >= 2:
            batch_math(0, bi=0)
            if SUB >= 2:
                sweep_A(0, zst[0], 0)
            if SUB >= 3:
                finalize_cv(0)
            if SUB >= 4:
                sweep_B(0, zst[0], zst[1], 1)
            if SUB >= 5:
                batch_math(1)
                sweep_F(0, zst[1], zst[2], 1, 2, bw[1]["wq"], last=False)
        if STAGE >= 3:
            batch_math(2, bi=1)
            sweep_A(1, zst[2], 2)
            finalize_cv(1)
            sweep_B(1, zst[2], zst[3], 3)
            batch_math(3)
            sweep_F(1, zst[3], None, 3, None, None, last=True)

    nc.compile()
    return nc


_NC = None


def _get_nc():
    global _NC
    if _NC is None:
        _NC = build()
    return _NC


def _prep(inputs):
    f32 = lambda a: np.ascontiguousarray(np.asarray(a, np.float32))
    bf = lambda a: np.ascontiguousarray(np.asarray(a, np.float32)).astype(NPBF16)
    dw = f32(inputs["dw_w"]).reshape(C, 27)
    base = {
        "dwTap": np.ascontiguousarray(dw.reshape(2, 128, 27)),
        "dwB": np.ascontiguousarray(f32(inputs["dw_b"]).reshape(2, 128).T),
        "eye": np.eye(128, dtype=np.float32).astype(NPBF16),
        "pw1W": bf(inputs["pw1_w"]),
        "pw1B": np.ascontiguousarray(f32(inputs["pw1_b"]).reshape(3, 128).T),
        "pw2W": bf(inputs["pw2_w"]),
        "pw2B": np.ascontiguousarray(f32(inputs["pw2_b"]).reshape(2, 128).T),
    }
    diag = np.zeros((max(1, len(PE_TAPS)), 2, 128, 128), np.float32)
    for ti, tp in enumerate(PE_TAPS):
        for cti in range(2):
            np.fill_diagonal(diag[ti, cti], dw[cti * 128 : (cti + 1) * 128, tp])
    base["dwDiag"] = diag.astype(NPBF16)
    qf = np.zeros((NBLK, 1), np.float32)
    for i in range(NBLK):
        qkvW = f32(inputs["ln1_g"][i])[:, None] * f32(inputs["qkv_w"][i])
        qkvB = f32(inputs["ln1_b"][i]) @ f32(inputs["qkv_w"][i]) + f32(inputs["qkv_b"][i])
        ff1W = f32(inputs["ln2_g"][i])[:, None] * f32(inputs["ff1_w"][i])
        ff1B = f32(inputs["ln2_b"][i]) @ f32(inputs["ff1_w"][i]) + f32(inputs["ff1_b"][i])
        wqb = np.ascontiguousarray(qkvW[:, 0:1]).astype(NPBF16)
        qf[i, 0] = -float(np.asarray(wqb, np.float32).sum())
        base.update({
            f"wq{i}": wqb,
            f"wk{i}": np.ascontiguousarray(qkvW[:, 1 : 1 + D]).astype(NPBF16),
            f"wv{i}": np.ascontiguousarray(qkvW[:, 1 + D :]).astype(NPBF16),
            f"qB{i}": np.ascontiguousarray(qkvB[0:1].reshape(1, 1)),
            f"kB{i}": np.ascontiguousarray(qkvB[1 : 1 + D].reshape(3, 128).T),
            f"vB{i}": np.ascontiguousarray(qkvB[1 + D :].reshape(3, 128).T),
            f"woW{i}": bf(inputs["wo_w"][i]),
            f"woB{i}": np.ascontiguousarray(f32(inputs["wo_b"][i]).reshape(3, 128).T),
            f"ff1W{i}": ff1W.astype(NPBF16),
            f"ff1B{i}": np.ascontiguousarray(ff1B.reshape(6, 128).T),
            f"ff2W{i}": bf(inputs["ff2_w"][i]),
            f"ff2B{i}": np.ascontiguousarray(f32(inputs["ff2_b"][i]).reshape(3, 128).T),
        })
    base["qfix"] = qf
    return base


def kernel(**inputs):
    base = _prep(inputs)
    x = np.asarray(inputs["x"], np.float32)
    in_maps = []
    for b in range(B):
        xb = np.ascontiguousarray(x[b].reshape(2, 128, T, H * W)).astype(NPBF16)
        in_maps.append(dict(base, xbf=xb))
    nc = _get_nc()
    trace = bool(int(os.environ.get("KERNEL_TRACE", "0")))
    res = run_bass_kernel_spmd(nc, in_maps, list(range(B)), trace=trace)
    kernel.last_exec_ns = res.exec_time_ns
    kernel.last_profile = res.profile_json
    kernel.last_results = res.results
    outs = [res.results[b]["out"].reshape(OUTC, T, H, W) for b in range(B)]
    return np.stack(outs).astype(np.float32)


# revision 13
# speedup vs baseline: 1.1175x; 1.0451x over previous
"""MobileViTV2 block kernel v2 — 8 TRN2 cores, data-parallel over batch.

All-bf16 datapath (fp32 PSUM), CH=1024 t-slice chunks, patch-major token
order j = p*256 + hp*16 + wp inside each chunk. z streams through DRAM in
bf16 between sweeps. Per block:
  sweep_A: normalize -> v GEMM -> relu -> spill v; zsum += cs*zn (ttr)
     [k-GEMM eliminated: cv = Wk^T zsum * (1/Z) + kB]
  sweep_B: per-patch wo GEMMs with cv FOLDED INTO wo weights; residual
     added via identity-matmul into PSUM; LN2 stats.
  sweep_F: FFN (+residual via identity-matmul) + next LN1 stats + q GEMM;
     last block variant does pw2 -> out (natural order restored in the
     PSUM->SBUF move).
LN stat math is batched: per-chunk stat rows (sum/sumsq/qraw via ones/wq
matmuls) bounce through DRAM into [16,1024] chunk-per-partition tiles so
the stat pipeline runs once per LN event, not once per chunk. R and M*R
rows bounce back via DMA broadcast reads. Conv taps split across PE
(diag-matmul), DVE and GPSIMD (env-tunable).
"""

import sys

sys.path.insert(0, "/opt/trn_rl_repo")
import os
import numpy as np
from contextlib import ExitStack

import concourse.bass as bass
import concourse.mybir as mybir
import concourse.tile as tile
from concourse import bacc
from concourse.bass_utils import run_bass_kernel_spmd

F32 = mybir.dt.float32
F32R = mybir.dt.float32r
BF16 = mybir.dt.bfloat16
AF = mybir.ActivationFunctionType
OP = mybir.AluOpType
NPBF16 = mybir.dt.np(BF16)

B, C, T, H, W = 8, 256, 16, 32, 32
D, OUTC, NBLK, FF = 384, 256, 2, 768
NTOK = T * H * W
CH = 1024
NCH = 16
PWD = 34
PSL = PWD * PWD
EPS = 1e-5

STAGE = int(os.environ.get("KERNEL_STAGE", "3"))
SUB = int(os.environ.get("KERNEL_SUB", "9"))
SIM_SAFE = bool(int(os.environ.get("KERNEL_SIM_SAFE", "0")))
N_PE_TAPS = int(os.environ.get("KERNEL_PE_TAPS", "27"))
N_GPS_TAPS = int(os.environ.get("KERNEL_GPS_TAPS", "0"))
ALL_TAPS = list(range(27))
PE_TAPS = ALL_TAPS[:N_PE_TAPS]
GPS_TAPS = ALL_TAPS[N_PE_TAPS : N_PE_TAPS + N_GPS_TAPS]
DVE_TAPS = ALL_TAPS[N_PE_TAPS + N_GPS_TAPS :]

SILU = AF.Square if SIM_SAFE else AF.Silu
EXP = AF.Square if SIM_SAFE else AF.Exp


def rawap(base, dims):
    return bass.AP(tensor=base.tensor, offset=base.offset, ap=[base.ap[0]] + dims)


def bcast_row(row, n, cols):
    """DRAM row -> [n, cols] broadcast-read AP."""
    return bass.AP(tensor=row.tensor, offset=row.offset, ap=[[0, n], [1, cols]])


# patch-major <-> natural permute views (within one 1024-token t-slice)
def nat2pm_in(ps):
    return ps.rearrange("p (hp ph wp pw) -> p ph pw hp wp", hp=16, ph=2, wp=16, pw=2)


def nat2pm_out(zslice):
    return zslice.rearrange("p (ph pw hp wp) -> p ph pw hp wp", ph=2, pw=2, hp=16, wp=16)


def pm2nat_in(ps):
    return ps.rearrange("p (ph pw hp wp) -> p ph hp wp pw", ph=2, pw=2, hp=16, wp=16)


def pm2nat_out(oslice):
    return oslice.rearrange("p (hp ph wp pw) -> p ph hp wp pw", hp=16, ph=2, wp=16, pw=2)


def build():
    nc = bacc.Bacc("TRN2", target_bir_lowering=False, debug=False, num_devices=8)

    xbf = nc.dram_tensor("xbf", [2, 128, T, H * W], BF16, kind="ExternalInput").ap()
    dwTap = nc.dram_tensor("dwTap", [2, 128, 27], F32, kind="ExternalInput").ap()
    dwBd = nc.dram_tensor("dwB", [128, 2], F32, kind="ExternalInput").ap()
    dwDiag = nc.dram_tensor("dwDiag", [max(1, len(PE_TAPS)), 2, 128, 128], BF16, kind="ExternalInput").ap()
    eyeD = nc.dram_tensor("eye", [128, 128], BF16, kind="ExternalInput").ap()
    pw1W = nc.dram_tensor("pw1W", [C, D], BF16, kind="ExternalInput").ap()
    pw1B = nc.dram_tensor("pw1B", [128, 3], F32, kind="ExternalInput").ap()
    pw2W = nc.dram_tensor("pw2W", [D, OUTC], BF16, kind="ExternalInput").ap()
    pw2B = nc.dram_tensor("pw2B", [128, 2], F32, kind="ExternalInput").ap()
    blk = []
    for i in range(NBLK):
        blk.append(dict(
            wq=nc.dram_tensor(f"wq{i}", [D, 1], BF16, kind="ExternalInput").ap(),
            wk=nc.dram_tensor(f"wk{i}", [D, D], BF16, kind="ExternalInput").ap(),
            wv=nc.dram_tensor(f"wv{i}", [D, D], BF16, kind="ExternalInput").ap(),
            qB=nc.dram_tensor(f"qB{i}", [1, 1], F32, kind="ExternalInput").ap(),
            kB=nc.dram_tensor(f"kB{i}", [128, 3], F32, kind="ExternalInput").ap(),
            vB=nc.dram_tensor(f"vB{i}", [128, 3], F32, kind="ExternalInput").ap(),
            woW=nc.dram_tensor(f"woW{i}", [D, D], BF16, kind="ExternalInput").ap(),
            woB=nc.dram_tensor(f"woB{i}", [128, 3], F32, kind="ExternalInput").ap(),
            ff1W=nc.dram_tensor(f"ff1W{i}", [D, FF], BF16, kind="ExternalInput").ap(),
            ff1B=nc.dram_tensor(f"ff1B{i}", [128, 6], F32, kind="ExternalInput").ap(),
            ff2W=nc.dram_tensor(f"ff2W{i}", [FF, D], BF16, kind="ExternalInput").ap(),
            ff2B=nc.dram_tensor(f"ff2B{i}", [128, 3], F32, kind="ExternalInput").ap(),
        ))
    qfix = nc.dram_tensor("qfix", [NBLK, 1], F32, kind="ExternalInput").ap()

    out = nc.dram_tensor("out", [OUTC, NTOK], F32, kind="ExternalOutput").ap()
    zst = [nc.dram_tensor(n, [3, 128, NTOK], BF16, kind="ExternalOutput").ap()
           for n in ("z0", "zm0", "z1", "zm1")]
    vd = [nc.dram_tensor(f"v{i}", [3, 128, NTOK], BF16).ap() for i in range(NBLK)]
    statd = [nc.dram_tensor(f"statd{e}", [3, NCH, CH], BF16).ap() for e in range(4)]
    rd = [nc.dram_tensor(f"rd{e}", [2, NCH, CH], BF16).ap() for e in range(4)]
    csd = [nc.dram_tensor(f"csd{i}", [NCH, CH], BF16).ap() for i in range(NBLK)]

    with ExitStack() as ctx:
        tc = ctx.enter_context(tile.TileContext(nc))
        wpool = ctx.enter_context(tc.tile_pool(name="w", bufs=1))
        sp = ctx.enter_context(tc.tile_pool(name="s", bufs=2))
        pp = ctx.enter_context(tc.tile_pool(name="ps", bufs=3, space="PSUM"))
        cvp = ctx.enter_context(tc.tile_pool(name="cv", bufs=1))
        p1 = ctx.enter_context(tc.tile_pool(name="p1", bufs=2))

        def wt(name, dram, kdim, mdim, dt=BF16):
            tiles = []
            for ki in range((kdim + 127) // 128):
                t = wpool.tile([128, mdim], dt, tag=f"{name}{ki}")
                nc.sync.dma_start(out=t[:], in_=dram[ki * 128 : (ki + 1) * 128, :])
                tiles.append(t)
            return tiles

        def ftile(name, dram, cols):
            t = wpool.tile([128, cols], F32, tag=name)
            nc.sync.dma_start(out=t[:], in_=dram)
            return t

        dwT = wpool.tile([128, 2, 27], F32, tag="dwT")
        for cti in range(2):
            nc.sync.dma_start(out=dwT[:, cti, :], in_=dwTap[cti])
        dwb_t = ftile("dwB", dwBd, 2)
        diag_t = None
        if PE_TAPS:
            diag_t = wpool.tile([128, len(PE_TAPS), 2, 128], BF16, tag="diag")
            for ti in range(len(PE_TAPS)):
                for cti in range(2):
                    nc.sync.dma_start(out=diag_t[:, ti, cti, :], in_=dwDiag[ti, cti])
        eye_t = wpool.tile([128, 128], BF16, tag="eye")
        nc.sync.dma_start(out=eye_t[:], in_=eyeD)
        pw1_t = wt("pw1", pw1W, C, D)
        pw1b_t = ftile("pw1B", pw1B, 3)
        pw2_t = wt("pw2", pw2W, D, OUTC)
        pw2b_t = ftile("pw2B", pw2B, 2)
        bw = []
        for i in range(NBLK):
            bw.append(dict(
                wq=wt(f"wq{i}_", blk[i]["wq"], D, 1),
                wk=wt(f"wk{i}_", blk[i]["wk"], D, D),
                wv=wt(f"wv{i}_", blk[i]["wv"], D, D),
                kB=ftile(f"kB{i}", blk[i]["kB"], 3),
                vB=ftile(f"vB{i}", blk[i]["vB"], 3),
                wo=wt(f"wo{i}_", blk[i]["woW"], D, D),
                woB=ftile(f"woB{i}", blk[i]["woB"], 3),
                ff1=wt(f"ff1{i}_", blk[i]["ff1W"], D, FF),
                ff1B=ftile(f"ff1B{i}", blk[i]["ff1B"], 6),
                ff2=wt(f"ff2{i}_", blk[i]["ff2W"], FF, D),
                ff2B=ftile(f"ff2B{i}", blk[i]["ff2B"], 3),
            ))
        ones_bf = wpool.tile([128, 1], BF16, tag="ones_bf")
        nc.vector.memset(ones_bf[:], 1.0)
        ones16 = wpool.tile([16, 1], F32, tag="ones16")
        nc.vector.memset(ones16[:], 1.0)
        ones1r = wpool.tile([1, 128], F32, tag="ones1r")
        nc.vector.memset(ones1r[:], 1.0)
        eps16 = wpool.tile([16, 1], F32, tag="eps16")
        nc.vector.memset(eps16[:], EPS)
        qb16 = [wpool.tile([16, 1], F32, tag=f"qb16_{i}", name=f"qb16_{i}") for i in range(NBLK)]
        sq16 = [wpool.tile([16, 1], F32, tag=f"sq16_{i}", name=f"sq16_{i}") for i in range(NBLK)]
        for i in range(NBLK):
            nc.sync.dma_start(out=qb16[i][:], in_=bcast_row(blk[i]["qB"][0, :], 16, 1))
            nc.sync.dma_start(out=sq16[i][:], in_=bcast_row(qfix[i, :], 16, 1))

        wop = [cvp.tile([128, 4, 384], BF16, tag=f"wop{k}", name=f"wop{k}") for k in range(3)]
        att = []
        for i in range(NBLK):
            a = dict(
                cvacc=cvp.tile([128, 3, 4], F32, tag=f"cvacc{i}", name=f"cvacc{i}"),
                zinvb=cvp.tile([128, 4], F32, tag=f"zinvb{i}", name=f"zinvb{i}"),
                wop=wop,
            )
            nc.vector.memset(a["cvacc"][:], 0.0)
            att.append(a)

        # ---------- helpers ----------
        def ln_stats(zt, wq_tiles, event, chunk, zsq_on_act):
            zsq = sp.tile([128, 3, CH], BF16, tag="zsq", name="zsq", bufs=1)
            if zsq_on_act:
                nc.scalar.activation(out=zsq[:], in_=zt[:], func=AF.Square)
            else:
                nc.vector.tensor_mul(zsq[:], zt[:], zt[:])
            ps = pp.tile([128, CH], F32, tag="pstat", bufs=1)
            for half in range(2):
                hsl = slice(half * 512, (half + 1) * 512)
                for kt in range(3):
                    nc.tensor.matmul(ps[0:1, hsl], ones_bf[:], zt[:, kt, hsl], start=(kt == 0), stop=(kt == 2))
                    nc.tensor.matmul(ps[32:33, hsl], ones_bf[:], zsq[:, kt, hsl], start=(kt == 0), stop=(kt == 2))
                    if wq_tiles is not None:
                        nc.tensor.matmul(ps[64:65, hsl], wq_tiles[kt][:], zt[:, kt, hsl], start=(kt == 0), stop=(kt == 2))
            sb = sp.tile([65, CH], BF16, tag="sbstat", name="sbstat", bufs=2)
            nc.vector.tensor_copy(sb[:], ps[0:65, :])
            nc.sync.dma_start(out=statd[event][0, chunk, :], in_=sb[0:1, :])
            nc.sync.dma_start(out=statd[event][1, chunk, :], in_=sb[32:33, :])
            if wq_tiles is not None:
                nc.sync.dma_start(out=statd[event][2, chunk, :], in_=sb[64:65, :])

        def batch_math(event, bi=None):
            with nc.allow_low_precision(reason="LN stat math in bf16 is within error budget"):
                return _batch_math(event, bi)

        def _batch_math(event, bi=None):
            bs = sp.tile([16, 3, CH], BF16, tag="bs", name="bs", bufs=1)
            for s in range(3 if bi is not None else 2):
                nc.sync.dma_start(out=bs[:, s, :], in_=statd[event][s])
            tmp = sp.tile([16, 5, CH], BF16, tag="bstmp", name="bstmp", bufs=1)
            M, t2, R, MR, q1 = (tmp[:, j, :] for j in range(5))
            nc.vector.tensor_scalar_mul(out=M, in0=bs[:, 0, :], scalar1=1.0 / D)
            nc.vector.tensor_mul(t2, M, M)
            nc.vector.scalar_tensor_tensor(out=t2, in0=bs[:, 1, :], scalar=1.0 / D, in1=t2, op0=OP.mult, op1=OP.subtract)
            nc.scalar.activation(out=t2, in_=t2, func=AF.Sqrt, bias=eps16[:])
            nc.vector.reciprocal(R, t2)
            nc.vector.tensor_mul(MR, M, R)
            rb = sp.tile([16, 2, CH], BF16, tag="rbf", name="rbf", bufs=1)
            nc.vector.tensor_copy(rb[:, 0, :], R)
            nc.vector.tensor_copy(rb[:, 1, :], MR)
            nc.sync.dma_start(out=rd[event][0], in_=rb[:, 0, :])
            nc.sync.dma_start(out=rd[event][1], in_=rb[:, 1, :])
            if bi is None:
                return
            nc.vector.scalar_tensor_tensor(out=q1, in0=M, scalar=sq16[bi][:], in1=bs[:, 2, :], op0=OP.mult, op1=OP.add)
            nc.vector.tensor_mul(q1, q1, R)
            cs = sp.tile([16, CH], BF16, tag="bscs", name="bscs", bufs=1)
            zp = sp.tile([16, 4], F32, tag="bszp", name="bszp", bufs=1)
            for p in range(4):
                nc.scalar.activation(out=cs[:, p * 256 : (p + 1) * 256], in_=q1[:, p * 256 : (p + 1) * 256],
                                     func=EXP, bias=qb16[bi][:], accum_out=zp[:, p : p + 1])
            nc.sync.dma_start(out=csd[bi], in_=cs[:])
            psz = pp.tile([128, CH], F32, tag="pstat", bufs=1)
            nc.tensor.matmul(psz[0:1, 0:4], ones16[:], zp[:], start=True, stop=True)
            zi = sp.tile([1, 4], F32, tag="zi", name="zi", bufs=1)
            nc.vector.reciprocal(zi[:], psz[0:1, 0:4])
            psb = pp.tile([128, CH], F32, tag="pstat", bufs=1)
            nc.tensor.matmul(psb[:, 0:4], ones1r[:], zi[:], start=True, stop=True)
            nc.vector.tensor_copy(att[bi]["zinvb"][:], psb[:, 0:4])

        def load_bcast(dram_row):
            t = sp.tile([128, CH], BF16, tag="bcast", name="bcast", bufs=3)
            nc.sync.dma_start(out=t[:], in_=bcast_row(dram_row, 128, CH))
            return t

        def load_z(dram, chunk, tag="zch"):
            zt = sp.tile([128, 3, CH], BF16, tag=tag, name=tag, bufs=2)
            for m in range(3):
                nc.sync.dma_start(out=zt[:, m, :], in_=dram[m, :, chunk * CH : (chunk + 1) * CH])
            return zt

        def normalize(zt, event, chunk, gps=False):
            rb = load_bcast(rd[event][0, chunk, :])
            mrb = load_bcast(rd[event][1, chunk, :])
            zn = sp.tile([128, 3, CH], BF16, tag="zn", name="zn", bufs=2)
            nc.vector.tensor_mul(zn[:], zt[:], rawap(rb[:], [[0, 3], [1, CH]]))
            eng = nc.gpsimd if gps else nc.vector
            eng.tensor_sub(zn[:], zn[:], rawap(mrb[:], [[0, 3], [1, CH]]))
            return zn

        # ================= S0: conv + pw1 + LN1_0 stats + q0 =================
        xslices = {}
        # zero the 3 rotating xps buffers once; interiors are overwritten by
        # each slice DMA, pad borders stay zero across rotations
        for _i in range(3):
            xz = p1.tile([128, 2, PSL], BF16, tag="xps", name="xps", bufs=3)
            nc.vector.memset(xz[:], 0.0)

        def load_slice(ts_):
            xs = p1.tile([128, 2, PSL], BF16, tag="xps", name="xps", bufs=3)
            for cti in range(2):
                dst = xs[:, cti, :].rearrange("p (h w) -> p h w", h=PWD)
                nc.sync.dma_start(out=dst[:, 1:33, 1:33], in_=xbf[cti, :, ts_, :].rearrange("p (h w) -> p h w", h=H))
            xslices[ts_] = xs

        for t in range(T):
            for ts_ in (t - 1, t, t + 1):
                if 0 <= ts_ < T and ts_ not in xslices:
                    load_slice(ts_)
            yact = p1.tile([128, 2, CH], BF16, tag="yact", name="yact")
            for cti in range(2):
                ok = lambda tp: 0 <= t + tp // 9 - 1 < T
                pe_here = [tp for tp in PE_TAPS if ok(tp)]
                dve_here = [tp for tp in DVE_TAPS if ok(tp)]
                gps_here = [tp for tp in GPS_TAPS if ok(tp)]

                def xsrc(tp):
                    dt_, dh, dw = tp // 9, (tp % 9) // 3, tp % 3
                    xv = xslices[t + dt_ - 1][:, cti, :].rearrange("p (h w) -> p h w", h=PWD)
                    return xv[:, dh : dh + 32, dw : dw + 32]

                ps_c = None
                if pe_here:
                    ps_c = pp.tile([128, CH], F32, tag="psA")
                    for half in range(2):
                        for j, tp in enumerate(pe_here):
                            dt_, dh, dw = tp // 9, (tp % 9) // 3, tp % 3
                            xv = xslices[t + dt_ - 1][:, cti, :].rearrange("p (h w) -> p h w", h=PWD)
                            nc.tensor.matmul(ps_c[:, half * 512 : (half + 1) * 512],
                                             diag_t[:, PE_TAPS.index(tp), cti, :],
                                             xv[:, dh + 16 * half : dh + 16 * half + 16, dw : dw + 32],
                                             start=(j == 0), stop=(j == len(pe_here) - 1))
                silu_src = ps_c[:] if ps_c is not None else None
                if dve_here or gps_here:
                    acc = p1.tile([128, CH], F32, tag="cacc", name="cacc", bufs=2)
                    accv = acc[:].rearrange("p (h w) -> p h w", h=H)
                    first = True
                    for tp in dve_here:
                        wcol = dwT[:, cti, tp : tp + 1]
                        if first:
                            nc.vector.tensor_scalar_mul(out=accv, in0=xsrc(tp), scalar1=wcol)
                        else:
                            nc.vector.scalar_tensor_tensor(out=accv, in0=xsrc(tp), scalar=wcol, in1=accv, op0=OP.mult, op1=OP.add)
                        first = False
                    gacc = None
                    for gj, tp in enumerate(gps_here):
                        wcol = dwT[:, cti, tp : tp + 1]
                        if gj == 0:
                            gacc = p1.tile([128, CH], F32, tag="gacc", name="gacc", bufs=1)
                            gaccv = gacc[:].rearrange("p (h w) -> p h w", h=H)
                            nc.gpsimd.tensor_scalar_mul(out=gaccv, in0=xsrc(tp), scalar1=wcol)
                        else:
                            nc.gpsimd.scalar_tensor_tensor(out=gaccv, in0=xsrc(tp), scalar=wcol, in1=gaccv, op0=OP.mult, op1=OP.add)
                    if gacc is not None and not first:
                        nc.vector.tensor_add(acc[:], acc[:], gacc[:])
                    elif gacc is not None:
                        acc = gacc
                    if ps_c is not None:
                        nc.vector.scalar_tensor_tensor(out=acc[:], in0=ps_c[:], scalar=1.0, in1=acc[:], op0=OP.mult, op1=OP.add)
                    silu_src = acc[:]
                nc.scalar.activation(out=yact[:, cti, :], in_=silu_src, func=SILU, bias=dwb_t[:, cti : cti + 1])
            zt = sp.tile([128, 3, CH], BF16, tag="zch", name="zch", bufs=2)
            for m in range(3):
                ps1 = pp.tile([128, CH], F32, tag="psA")
                for half in range(2):
                    hsl = slice(half * 512, (half + 1) * 512)
                    for kt in range(2):
                        nc.tensor.matmul(ps1[:, hsl], pw1_t[kt][:, m * 128 : (m + 1) * 128], yact[:, kt, hsl], start=(kt == 0), stop=(kt == 1))
                for ph_ in range(2):
                    nc.scalar.activation(out=nat2pm_out(zt[:, m, :])[:, ph_], in_=nat2pm_in(ps1[:])[:, ph_], func=AF.Identity, bias=pw1b_t[:, m : m + 1])
                nc.sync.dma_start(out=zst[0][m, :, t * CH : (t + 1) * CH], in_=zt[:, m, :])
            ln_stats(zt, bw[0]["wq"], 0, t, zsq_on_act=True)

        # ================= per-block sweeps =================
        def sweep_A(bi, zsrc, event):
            a = att[bi]
            for chunk in range(NCH):
                zt = load_z(zsrc, chunk)
                zn = normalize(zt, event, chunk)
                csb = load_bcast(csd[bi][chunk, :])
                vt = sp.tile([128, 3, CH], BF16, tag="vch", name="vch", bufs=2)
                for m in range(3):
                    psv = pp.tile([128, CH], F32, tag="psA")
                    for half in range(2):
                        hsl = slice(half * 512, (half + 1) * 512)
                        for kt in range(3):
                            nc.tensor.matmul(psv[:, hsl], bw[bi]["wv"][kt][:, m * 128 : (m + 1) * 128], zn[:, kt, hsl], start=(kt == 0), stop=(kt == 2))
                    nc.scalar.activation(out=vt[:, m, :], in_=psv[:], func=AF.Relu, bias=bw[bi]["vB"][:, m : m + 1])
                    nc.sync.dma_start(out=vd[bi][m, :, chunk * CH : (chunk + 1) * CH], in_=vt[:, m, :])
                junk = sp.tile([128, 3, CH], BF16, tag="junk", name="junk", bufs=2)
                csb3 = rawap(csb[:], [[0, 3], [1, CH]])
                nc.vector.tensor_mul(junk[:], zn[:], csb3)
                cvch = sp.tile([128, 3, 4], F32, tag="cvch", name="cvch", bufs=2)
                for m in range(3):
                    for p in range(4):
                        if (m + p) % 2 == 0:
                            nc.vector.tensor_reduce(cvch[:, m, p : p + 1], junk[:, m, p * 256 : (p + 1) * 256],
                                                    axis=mybir.AxisListType.X, op=OP.add)
                        else:
                            nc.scalar.activation(out=junk[:, m, p * 256 : (p + 1) * 256], in_=junk[:, m, p * 256 : (p + 1) * 256],
                                                 func=AF.Copy, accum_out=cvch[:, m, p : p + 1])
                nc.vector.tensor_add(a["cvacc"][:], a["cvacc"][:], cvch[:])

        def finalize_cv(bi):
            a = att[bi]
            cvb = sp.tile([128, 3, 4], BF16, tag="cvb", name="cvb", bufs=1)
            nc.vector.tensor_copy(cvb[:], a["cvacc"][:])
            cvf = sp.tile([128, 3, 4], F32, tag="cvf", name="cvf", bufs=1)
            for m in range(3):
                psc = pp.tile([128, CH], F32, tag="pstat", bufs=1)
                for kt in range(3):
                    nc.tensor.matmul(psc[:, 0:4], bw[bi]["wk"][kt][:, m * 128 : (m + 1) * 128], cvb[:, kt, :], start=(kt == 0), stop=(kt == 2))
                nc.vector.scalar_tensor_tensor(out=cvf[:, m, :], in0=psc[:, 0:4], scalar=bw[bi]["kB"][:, m : m + 1], in1=a["zinvb"][:], op0=OP.add, op1=OP.mult)
            for kt in range(3):
                for p in range(4):
                    nc.vector.tensor_scalar_mul(out=a["wop"][kt][:, p, :], in0=bw[bi]["wo"][kt][:], scalar1=cvf[:, kt, p : p + 1])

        def sweep_B(bi, zsrc, zdst, event):
            a = att[bi]
            for chunk in range(NCH):
                zt = load_z(zsrc, chunk)
                vt = sp.tile([128, 3, CH], BF16, tag="vch", name="vch", bufs=2)
                for m in range(3):
                    nc.sync.dma_start(out=vt[:, m, :], in_=vd[bi][m, :, chunk * CH : (chunk + 1) * CH])
                zm = sp.tile([128, 3, CH], BF16, tag="zm", name="zm", bufs=2)
                for m in range(3):
                    pso = pp.tile([128, CH], F32, tag="psA")
                    for p in range(4):
                        sl = slice(p * 256, (p + 1) * 256)
                        for kt in range(3):
                            nc.tensor.matmul(pso[:, sl], a["wop"][kt][:, p, m * 128 : (m + 1) * 128], vt[:, kt, sl], start=(kt == 0), stop=False)
                        nc.tensor.matmul(pso[:, sl], eye_t[:], zt[:, m, sl], start=False, stop=True)
                    nc.scalar.activation(out=zm[:, m, :], in_=pso[:], func=AF.Identity, bias=bw[bi]["woB"][:, m : m + 1])
                    nc.sync.dma_start(out=zdst[m, :, chunk * CH : (chunk + 1) * CH], in_=zm[:, m, :])
                ln_stats(zm, None, event, chunk, zsq_on_act=False)

        def sweep_F(bi, zsrc, zdst, event_in, event_out, wq_next, last):
            for chunk in range(NCH):
                zt = load_z(zsrc, chunk)
                zn = normalize(zt, event_in, chunk)
                z2 = sp.tile([128, 3, CH], BF16, tag="z2", name="z2", bufs=2)
                ht = sp.tile([128, 6, CH], BF16, tag="ht", name="ht", bufs=1)
                for m6 in range(6):
                    ps1 = pp.tile([128, CH], F32, tag="psA")
                    for half in range(2):
                        hsl = slice(half * 512, (half + 1) * 512)
                        for kt in range(3):
                            nc.tensor.matmul(ps1[:, hsl], bw[bi]["ff1"][kt][:, m6 * 128 : (m6 + 1) * 128], zn[:, kt, hsl], start=(kt == 0), stop=(kt == 2))
                    nc.scalar.activation(out=ht[:, m6, :], in_=ps1[:], func=SILU, bias=bw[bi]["ff1B"][:, m6 : m6 + 1])
                for m in range(3):
                    psf = pp.tile([128, CH], F32, tag="psA")
                    for half in range(2):
                        hsl = slice(half * 512, (half + 1) * 512)
                        for m6 in range(6):
                            nc.tensor.matmul(psf[:, hsl], bw[bi]["ff2"][m6][:, m * 128 : (m + 1) * 128], ht[:, m6, hsl], start=(m6 == 0), stop=False)
                        nc.tensor.matmul(psf[:, hsl], eye_t[:], zt[:, m, hsl], start=False, stop=True)
                    nc.scalar.activation(out=z2[:, m, :], in_=psf[:], func=AF.Identity, bias=bw[bi]["ff2B"][:, m : m + 1])
                    if zdst is not None:
                        nc.sync.dma_start(out=zdst[m, :, chunk * CH : (chunk + 1) * CH], in_=z2[:, m, :])
                if not last:
                    ln_stats(z2, wq_next, event_out, chunk, zsq_on_act=False)
                else:
                    for m in range(2):
                        ot = sp.tile([128, CH], F32, tag="ot", name="ot", bufs=1)
                        ps2 = pp.tile([128, CH], F32, tag="pstat", bufs=1)
                        for half in range(2):
                            hsl = slice(half * 512, (half + 1) * 512)
                            for kt in range(3):
                                nc.tensor.matmul(ps2[:, hsl], pw2_t[kt][:, m * 128 : (m + 1) * 128], z2[:, kt, hsl], start=(kt == 0), stop=(kt == 2))
                        for ph_ in range(2):
                            nc.scalar.activation(out=pm2nat_out(ot[:])[:, ph_], in_=pm2nat_in(ps2[:])[:, ph_], func=AF.Identity, bias=pw2b_t[:, m : m + 1])
                        nc.sync.dma_start(out=out[m * 128 : (m + 1) * 128, chunk * CH : (chunk + 1) * CH], in_=ot[:])

        if STAGE >= 2:
            batch_math(0, bi=0)
            if SUB >= 2:
                sweep_A(0, zst[0], 0)
            if SUB >= 3:
                finalize_cv(0)
            if SUB >= 4:
                sweep_B(0, zst[0], zst[1], 1)
            if SUB >= 5:
                batch_math(1)
                sweep_F(0, zst[1], zst[2], 1, 2, bw[1]["wq"], last=False)
        if STAGE >= 3:
            batch_math(2, bi=1)
            sweep_A(1, zst[2], 2)
            finalize_cv(1)
            sweep_B(1, zst[2], zst[3], 3)
            batch_math(3)
            sweep_F(1, zst[3], None, 3, None, None, last=True)

    nc.compile()
    return nc


_NC = None


def _get_nc():
    global _NC
    if _NC is None:
        _NC = build()
    return _NC


def _prep(inputs):
    f32 = lambda a: np.ascontiguousarray(np.asarray(a, np.float32))
    bf = lambda a: np.ascontiguousarray(np.asarray(a, np.float32)).astype(NPBF16)
    dw = f32(inputs["dw_w"]).reshape(C, 27)
    base = {
        "dwTap": np.ascontiguousarray(dw.reshape(2, 128, 27)),
        "dwB": np.ascontiguousarray(f32(inputs["dw_b"]).reshape(2, 128).T),
        "eye": np.eye(128, dtype=np.float32).astype(NPBF16),
        "pw1W": bf(inputs["pw1_w"]),
        "pw1B": np.ascontiguousarray(f32(inputs["pw1_b"]).reshape(3, 128).T),
        "pw2W": bf(inputs["pw2_w"]),
        "pw2B": np.ascontiguousarray(f32(inputs["pw2_b"]).reshape(2, 128).T),
    }
    diag = np.zeros((max(1, len(PE_TAPS)), 2, 128, 128), np.float32)
    for ti, tp in enumerate(PE_TAPS):
        for cti in range(2):
            np.fill_diagonal(diag[ti, cti], dw[cti * 128 : (cti + 1) * 128, tp])
    base["dwDiag"] = diag.astype(NPBF16)
    qf = np.zeros((NBLK, 1), np.float32)
    for i in range(NBLK):
        qkvW = f32(inputs["ln1_g"][i])[:, None] * f32(inputs["qkv_w"][i])
        qkvB = f32(inputs["ln1_b"][i]) @ f32(inputs["qkv_w"][i]) + f32(inputs["qkv_b"][i])
        ff1W = f32(inputs["ln2_g"][i])[:, None] * f32(inputs["ff1_w"][i])
        ff1B = f32(inputs["ln2_b"][i]) @ f32(inputs["ff1_w"][i]) + f32(inputs["ff1_b"][i])
        wqb = np.ascontiguousarray(qkvW[:, 0:1]).astype(NPBF16)
        qf[i, 0] = -float(np.asarray(wqb, np.float32).sum())
        base.update({
            f"wq{i}": wqb,
            f"wk{i}": np.ascontiguousarray(qkvW[:, 1 : 1 + D]).astype(NPBF16),
            f"wv{i}": np.ascontiguousarray(qkvW[:, 1 + D :]).astype(NPBF16),
            f"qB{i}": np.ascontiguousarray(qkvB[0:1].reshape(1, 1)),
            f"kB{i}": np.ascontiguousarray(qkvB[1 : 1 + D].reshape(3, 128).T),
            f"vB{i}": np.ascontiguousarray(qkvB[1 + D :].reshape(3, 128).T),
            f"woW{i}": bf(inputs["wo_w"][i]),
            f"woB{i}": np.ascontiguousarray(f32(inputs["wo_b"][i]).reshape(3, 128).T),
            f"ff1W{i}": ff1W.astype(NPBF16),
            f"ff1B{i}": np.ascontiguousarray(ff1B.reshape(6, 128).T),
            f"ff2W{i}": bf(inputs["ff2_w"][i]),
            f"ff2B{i}": np.ascontiguousarray(f32(inputs["ff2_b"][i]).reshape(3, 128).T),
        })
    base["qfix"] = qf
    return base


def kernel(**inputs):
    base = _prep(inputs)
    x = np.asarray(inputs["x"], np.float32)
    in_maps = []
    for b in range(B):
        xb = np.ascontiguousarray(x[b].reshape(2, 128, T, H * W)).astype(NPBF16)
        in_maps.append(dict(base, xbf=xb))
    nc = _get_nc()
    trace = bool(int(os.environ.get("KERNEL_TRACE", "0")))
    res = run_bass_kernel_spmd(nc, in_maps, list(range(B)), trace=trace)
    kernel.last_exec_ns = res.exec_time_ns
    kernel.last_profile = res.profile_json
    kernel.last_results = res.results
    outs = [res.results[b]["out"].reshape(OUTC, T, H, W) for b in range(B)]
    return np.stack(outs).astype(np.float32)
